# revision 1
# baseline (speedup 1.0000x reference)
"""MPI compositing + homography warp kernel for Trainium2 (8 NeuronCores).

For each of P=32 fronto-parallel planes and S=4 source images: composite
per-plane channels (net transmittance T, accumulated-over acc, full-over
bro, source image src -> 10 channels), then bilinear-warp each (plane, src)
channel stack by a plane/source-dependent homography. Output (P, S, 10, H, W).

Structure exploited: the target->source homography here has identity
rotation and shared intrinsics, so sample coordinate ix depends only on x
and iy only on y.  The bilinear gather (zero padding) then factorizes
EXACTLY into two small banded matrices applied left/right:

    warped = Wy @ S @ Wx^T        per (plane, src, channel)

with per-tap validity folded into the weights.  Wy/Wx are built on the host
from the pose inputs and executed as PE matmuls (fp32r = full-rate fp32 at
N>=256).

Sharding: core = (s, h) in 4 sources x 2 output-row-halves.  The cross-plane
compositing scan is pointwise in pixels -> fully core-local, no collectives.

Per core:
  pass A (d=31..0): T-state suffix scan; warp T (ch 0) and src (ch 7..9)
  pass B (d=0..31): over-state forward scan; warp acc (ch 1..3) of plane d+2
  pass C (d=0..31): warp bro = overs[31] (ch 4..6)
Warp of one channel-plane: mm1 (U = Wy @ S, 2 accumulated K-chunks),
ScalarE copy U PSUM->SBUF, PE transpose (2x 128x128), DVE copy U^T
PSUM->SBUF, mm2 (F = (U^T).T @ Wx^T), ScalarE copy, DMA out.

Channel-image SBUF layout: [128, 2*W]; image row r lives at partition
r % 128, columns (r // 128)*W ... +W.  This gives compositing a single
full-width free dim and gives mm1 its two K-chunk row views for free.
"""

import os
import sys

import numpy as np

sys.path.insert(0, "/opt/trn_rl_repo")

P, S, H, W = 32, 4, 256, 256
NCORES = 8
NCH = 10

# Intermediate dtype for the transpose + mm2 chain: "f32" or "bf16".
INTER = os.environ.get("KERNEL_INTER_DT", "f32")


def _compute_sample_coords(mpi_planes, pose_tgt, intrins_src, intrins_tgt):
    """Exact reference math for sample coords, float64. -> ix, iy (P,S,H,W)."""
    Kinv = np.linalg.inv(intrins_tgt.astype(np.float64))
    gx, gy = np.meshgrid(
        np.arange(W, dtype=np.float64), np.arange(H, dtype=np.float64)
    )
    pix = np.stack([gx.ravel(), gy.ravel(), np.ones(H * W)])  # (3, HW)
    cam_dir = Kinv @ pix  # (3, HW)
    ix = np.empty((P, S, H, W))
    iy = np.empty((P, S, H, W))
    for s in range(S):
        K4 = np.zeros((4, 4))
        K4[:3, :3] = intrins_src[s].astype(np.float64)
        K4[3, 3] = 1.0
        proj = K4 @ pose_tgt[s].astype(np.float64)
        for p in range(P):
            cam = np.concatenate(
                [cam_dir * np.float64(mpi_planes[p]), np.ones((1, H * W))], 0
            )
            upc = proj @ cam
            z = upc[2] + 1e-10
            ix[p, s] = (upc[0] / z).reshape(H, W)
            iy[p, s] = (upc[1] / z).reshape(H, W)
    return ix, iy


def _bilinear_matrix(coord_1d, n_in):
    """1D resample matrix M[out, in] with reference tap/validity semantics."""
    n_out = coord_1d.shape[0]
    M = np.zeros((n_out, n_in), np.float64)
    c0 = np.floor(coord_1d)
    w1 = coord_1d - c0
    w0 = 1.0 - w1
    for o in range(n_out):
        i0 = int(c0[o])
        if 0 <= i0 <= n_in - 1:
            M[o, i0] += w0[o]
        if 0 <= i0 + 1 <= n_in - 1:
            M[o, i0 + 1] += w1[o]
    return M


def _reference_numpy(colors, alphas, imgs_src, mpi_planes, pose_tgt,
                     intrins_src, intrins_tgt):
    """Pure-numpy replica of the reference (generic fallback + self-test)."""
    Pn, Sn, Hh, Ww = alphas.shape
    ca = 1.0 - alphas
    pm = colors * alphas[..., None]
    overs = np.empty_like(pm)
    over = np.zeros_like(pm[0])
    for d in range(Pn):
        over = over * ca[d][..., None] + pm[d]
        overs[d] = over
    acc = overs[np.maximum(np.arange(Pn) - 2, 0)]
    bro = np.broadcast_to(overs[-1][None], (Pn, Sn, Hh, Ww, 3))
    rc = np.cumprod(ca[::-1], axis=0)[::-1]
    T = np.concatenate([rc[1:], np.ones_like(rc[:1])], axis=0)
    src = np.broadcast_to(imgs_src[None], (Pn, Sn, Hh, Ww, 3))
    stacked = np.concatenate([T[..., None], acc, bro, src], axis=-1)

    ix, iy = _compute_sample_coords(mpi_planes, pose_tgt, intrins_src,
                                    intrins_tgt)
    out = np.empty((Pn, Sn, NCH, Hh, Ww), np.float32)
    for p in range(Pn):
        for s in range(Sn):
            img = stacked[p, s]
            x0 = np.floor(ix[p, s])
            y0 = np.floor(iy[p, s])
            wx1 = ix[p, s] - x0
            wx0 = 1.0 - wx1
            wy1 = iy[p, s] - y0
            wy0 = 1.0 - wy1

            def gather(xx, yy):
                valid = (xx >= 0) & (xx <= Ww - 1) & (yy >= 0) & (yy <= Hh - 1)
                xc = np.clip(xx, 0, Ww - 1).astype(np.int64)
                yc = np.clip(yy, 0, Hh - 1).astype(np.int64)
                return img[yc, xc] * valid[..., None]

            warped = (gather(x0, y0) * (wx0 * wy0)[..., None]
                      + gather(x0 + 1, y0) * (wx1 * wy0)[..., None]
                      + gather(x0, y0 + 1) * (wx0 * wy1)[..., None]
                      + gather(x0 + 1, y0 + 1) * (wx1 * wy1)[..., None])
            out[p, s] = warped.transpose(2, 0, 1).astype(np.float32)
    return out


_CACHED = {}


def _build_bass_program():
    """Build (once) the SPMD Bass program shared by all 8 cores."""
    if "nc" in _CACHED:
        return _CACHED["nc"]

    import concourse.bacc as bacc
    import concourse.mybir as mybir
    from concourse import tile

    f32 = mybir.dt.float32
    f32r = mybir.dt.float32r
    bf16 = mybir.dt.bfloat16
    inter_dt = bf16 if INTER == "bf16" else f32r

    nc = bacc.Bacc(
        "TRN2", target_bir_lowering=False, debug=False,
        enable_asserts=False, num_devices=NCORES,
    )

    alphas_d = nc.dram_tensor("alphas", [P, H, W], f32, kind="ExternalInput").ap()
    colors_d = nc.dram_tensor("colors", [P, 3, H, W], f32, kind="ExternalInput").ap()
    src_d = nc.dram_tensor("src", [3, H, W], f32r, kind="ExternalInput").ap()
    wyt_d = nc.dram_tensor("wyt", [P, H, 128], f32r, kind="ExternalInput").ap()
    wxt_d = nc.dram_tensor("wxt", [P, 2, 128, W], inter_dt,
                           kind="ExternalInput").ap()
    ident_d = nc.dram_tensor("ident", [128, 128], inter_dt,
                             kind="ExternalInput").ap()
    zeros_d = nc.dram_tensor("zeros", [128, 2 * W], f32r,
                             kind="ExternalInput").ap()
    ones_d = nc.dram_tensor("ones", [128, 2 * W], f32r,
                            kind="ExternalInput").ap()
    out_d = nc.dram_tensor("out", [P, NCH, 128, W], f32,
                           kind="ExternalOutput").ap()

    with tile.TileContext(nc) as tc:
        with (
            tc.tile_pool(name="alpha", bufs=P) as alpha_pool,
            tc.tile_pool(name="persist", bufs=1) as persist,
            tc.tile_pool(name="wy", bufs=P) as wy_pool,
            tc.tile_pool(name="wx", bufs=4) as wx_pool,
            tc.tile_pool(name="cols", bufs=3) as cols_pool,
            tc.tile_pool(name="work", bufs=5) as work,
            tc.tile_pool(name="tmp", bufs=2) as tmpp,
            tc.tile_pool(name="psum", bufs=2, space="PSUM") as psum,
        ):
            ident_sb = persist.tile([128, 128], inter_dt, tag="ident", name="ident_sb")
            nc.sync.dma_start(ident_sb[:], ident_d[:])

            over_sb = [persist.tile([128, 2 * W], f32r, tag=f"over{c}", name=f"over_sb{c}")
                       for c in range(3)]
            t_sb = persist.tile([128, 2 * W], f32r, tag="tchan", name="t_sb")
            src_sb = [persist.tile([128, 2 * W], f32r, tag=f"src{c}", name=f"src_sb{c}")
                      for c in range(3)]
            for c in range(3):
                nc.sync.dma_start(over_sb[c][:], zeros_d[:])
                nc.sync.dma_start(
                    src_sb[c][:].rearrange("p (c w) -> p c w", c=2),
                    src_d[c].rearrange("(c p) w -> p c w", p=128),
                )
            nc.sync.dma_start(t_sb[:], ones_d[:])

            alpha_sb = [alpha_pool.tile([128, 2 * W], f32, tag="alpha", name="alpha_sb")
                        for _ in range(P)]
            wy_sb = [wy_pool.tile([128, 256], f32r, tag="wy", name="wy_sb") for _ in range(P)]
            for d in range(P):
                nc.sync.dma_start(
                    wy_sb[d][:].rearrange("p (c m) -> p c m", c=2),
                    wyt_d[d].rearrange("(c p) m -> p c m", p=128),
                )

            def load_wx(d):
                wx_t = wx_pool.tile([128, 2 * W], inter_dt, tag="wx", name="wx_t")
                nc.sync.dma_start(
                    wx_t[:].rearrange("p (c m) -> p c m", c=2),
                    wxt_d[d].rearrange("c p m -> p c m"),
                )
                return wx_t

            def warp(d, s_tile, ci, wx_t):
                """Warp channel image s_tile with plane-d matrices -> out[d,ci]."""
                U = psum.tile([128, W], f32, tag="U", name="U_ps")
                nc.tensor.matmul(
                    U[:], wy_sb[d][:, 0:128], s_tile[:, 0:W],
                    start=True, stop=False,
                )
                nc.tensor.matmul(
                    U[:], wy_sb[d][:, 128:256], s_tile[:, W:2 * W],
                    start=False, stop=True,
                )
                u_sb = work.tile([128, W], inter_dt, tag="u_sb", name="u_sb")
                nc.scalar.copy(u_sb[:], U[:])
                UT = psum.tile([128, W], inter_dt, tag="UT", name="UT_ps")
                nc.tensor.transpose(UT[:, 0:128], u_sb[:, 0:128], ident_sb[:])
                nc.tensor.transpose(UT[:, 128:256], u_sb[:, 128:256],
                                    ident_sb[:])
                ut_sb = work.tile([128, W], inter_dt, tag="ut_sb", name="ut_sb")
                nc.vector.tensor_copy(ut_sb[:], UT[:])
                F = psum.tile([128, W], f32, tag="F", name="F_ps")
                nc.tensor.matmul(F[:], ut_sb[:, 0:128], wx_t[:, 0:W],
                                 start=True, stop=False)
                nc.tensor.matmul(F[:], ut_sb[:, 128:256], wx_t[:, W:2 * W],
                                 start=False, stop=True)
                f_sb = work.tile([128, W], f32, tag="f_sb", name="f_sb")
                nc.scalar.copy(f_sb[:], F[:])
                nc.sync.dma_start(out_d[d, ci], f_sb[:])

            # ---- pass A: backward suffix scan for T; warp T + src ------
            for d in range(P - 1, -1, -1):
                nc.sync.dma_start(
                    alpha_sb[d][:].rearrange("p (c w) -> p c w", c=2),
                    alphas_d[d].rearrange("(c p) w -> p c w", p=128),
                )
                wx_t = load_wx(d)
                warp(d, t_sb, 0, wx_t)
                for c in range(3):
                    warp(d, src_sb[c], 7 + c, wx_t)
                # T <- T * (1 - alpha_d)   (ordered after the T warp's reads)
                tt = tmpp.tile([128, 2 * W], f32, tag="t_tmp", name="t_tmp")
                nc.gpsimd.tensor_mul(tt[:], t_sb[:], alpha_sb[d][:])
                nc.gpsimd.tensor_sub(t_sb[:], t_sb[:], tt[:])

            # ---- pass B: forward over scan; warp acc -------------------
            for d in range(P):
                col_t = [cols_pool.tile([128, 2 * W], f32, tag=f"col{c}", name=f"col_t{c}")
                         for c in range(3)]
                for c in range(3):
                    nc.sync.dma_start(
                        col_t[c][:].rearrange("p (c2 w) -> p c2 w", c2=2),
                        colors_d[d, c].rearrange("(c2 p) w -> p c2 w", p=128),
                    )
                # over_c += alpha_d * (colors_c - over_c)
                for c in range(3):
                    eng = nc.vector if c < 2 else nc.gpsimd
                    t = tmpp.tile([128, 2 * W], f32, tag=f"ov_tmp{c}", name=f"ov_tmp{c}")
                    eng.tensor_sub(t[:], col_t[c][:], over_sb[c][:])
                    eng.tensor_mul(t[:], t[:], alpha_sb[d][:])
                    eng.tensor_add(over_sb[c][:], over_sb[c][:], t[:])
                # over == overs[d]; acc[pl] = overs[max(pl-2, 0)]
                if d == 0:
                    for pl in (0, 1, 2):
                        wx_t = load_wx(pl)
                        for c in range(3):
                            warp(pl, over_sb[c], 1 + c, wx_t)
                elif d <= P - 3:
                    wx_t = load_wx(d + 2)
                    for c in range(3):
                        warp(d + 2, over_sb[c], 1 + c, wx_t)

            # ---- pass C: warp bro = overs[-1] --------------------------
            for d in range(P):
                wx_t = load_wx(d)
                for c in range(3):
                    warp(d, over_sb[c], 4 + c, wx_t)

    nc.compile()
    _CACHED["nc"] = nc
    return nc


def _host_prepare(colors, alphas, imgs_src, mpi_planes, pose_tgt,
                  intrins_src, intrins_tgt):
    """Build per-core input maps. Returns (in_maps, separable)."""
    import ml_dtypes

    ix, iy = _compute_sample_coords(mpi_planes, pose_tgt, intrins_src,
                                    intrins_tgt)
    dev_x = np.abs(ix - ix[:, :, :1, :]).max()
    dev_y = np.abs(iy - iy[:, :, :, :1]).max()
    if dev_x > 1e-3 or dev_y > 1e-3:
        return None, False

    ix1 = ix[:, :, 0, :]  # (P, S, W)
    iy1 = iy[:, :, :, 0]  # (P, S, H)

    inter_np = ml_dtypes.bfloat16 if INTER == "bf16" else np.float32
    ident = np.eye(128, dtype=np.float32).astype(inter_np)

    in_maps = []
    for core in range(NCORES):
        s, h = divmod(core, 2)
        wyt = np.zeros((P, H, 128), np.float32)
        wxt = np.zeros((P, W, W), np.float32)
        for d in range(P):
            My = _bilinear_matrix(iy1[d, s, h * 128:(h + 1) * 128], H)
            wyt[d] = My.T.astype(np.float32)  # [yi, yo]
            Mx = _bilinear_matrix(ix1[d, s], W)
            wxt[d] = Mx.T.astype(np.float32)  # [xi, xo]
        in_maps.append({
            "alphas": np.ascontiguousarray(alphas[:, s]),
            "colors": np.ascontiguousarray(colors[:, s].transpose(0, 3, 1, 2)),
            "src": np.ascontiguousarray(imgs_src[s].transpose(2, 0, 1)),
            "wyt": wyt,
            "wxt": np.ascontiguousarray(
                wxt.reshape(P, 2, 128, W)).astype(inter_np),
            "ident": ident,
            "zeros": np.zeros((128, 2 * W), np.float32),
            "ones": np.ones((128, 2 * W), np.float32),
        })
    return in_maps, True


def kernel(colors, alphas, imgs_src, mpi_planes, pose_tgt, intrins_src,
           intrins_tgt):
    colors = np.asarray(colors, np.float32)
    alphas = np.asarray(alphas, np.float32)
    imgs_src = np.asarray(imgs_src, np.float32)
    mpi_planes = np.asarray(mpi_planes, np.float32)
    pose_tgt = np.asarray(pose_tgt, np.float32)
    intrins_src = np.asarray(intrins_src, np.float32)
    intrins_tgt = np.asarray(intrins_tgt, np.float32)

    in_maps, separable = _host_prepare(
        colors, alphas, imgs_src, mpi_planes, pose_tgt, intrins_src,
        intrins_tgt)
    if not separable:
        return _reference_numpy(colors, alphas, imgs_src, mpi_planes,
                                pose_tgt, intrins_src, intrins_tgt)

    from concourse.bass_utils import run_bass_kernel_spmd

    nc = _build_bass_program()
    res = run_bass_kernel_spmd(nc, in_maps, core_ids=list(range(NCORES)))
    _CACHED["last_results"] = res

    out = np.empty((P, S, NCH, H, W), np.float32)
    for core in range(NCORES):
        s, h = divmod(core, 2)
        out[:, s, :, h * 128:(h + 1) * 128, :] = res.results[core]["out"]
    return out



# revision 15
# speedup vs baseline: 1.1644x; 1.1644x over previous
"""MPI compositing + homography warp kernel for Trainium2 (8 NeuronCores).

For each of P=32 fronto-parallel planes and S=4 source images: composite
per-plane channels (net transmittance T, accumulated-over acc, full-over
bro, source image src -> 10 channels), then bilinear-warp each (plane, src)
channel stack by a plane/source-dependent homography. Output (P, S, 10, H, W).

Structure exploited: the target->source homography here has identity
rotation and shared intrinsics, so sample coordinate ix depends only on x
and iy only on y.  The bilinear gather (zero padding) then factorizes
EXACTLY into two small banded matrices applied left/right:

    warped = My @ S @ Mx^T        per (plane, src, channel)

with per-tap validity folded into the weights (built on the host from the
pose inputs).

Kernel architecture (v3), per core = (source s, row-half h):

1. Compositing as segmented scans.  Channel data lives in SBUF as
   [window-row partition, (ch, x, plane)] with plane minor, so the
   cross-plane recurrences run as `tensor_tensor_scan` along a contiguous
   free dim:
       over:  state = ca_d * state + pm_d      (ca zeroed at d=0 -> reset)
       T:     state = ca'_t * state + e_t      (reversed planes, e=1 at t=0)
   One scan instruction covers 32 planes x a 32-col x-block x 128 rows.

2. y-warp as "mm1T": U^T = (S_slice)^T @ Wy with the composited channel
   image as the matmul *stationary* operand (a stride-32 AP picking one
   plane) and Wy moving.  Produces the transposed intermediate without any
   PE transpose and with one PSUM->SBUF copy.  bro/src channels batch 4
   planes per matmul (shared stationary image, N=512).

3. x-warp: F^T chunks = wxt_block^T @ U^T with wxt stationary and
   channel-batched moving data (N<=512).  Output is written transposed
   ([xo, m, ch, yo]) and untransposed on the host.

Everything on-chip is bf16 except PSUM accumulation (f32); DRAM output is
bf16, upcast on the host.  The y-window (the ~128-160 input rows feeding a
core's 128 output rows) is computed from the actual pose at build time.
"""

import sys

import numpy as np

sys.path.insert(0, "/opt/trn_rl_repo")

P, S, H, W = 32, 4, 256, 256
NCORES = 8
NCH = 10
NPHASE = 8          # x-blocks for compositing DMA/scans
XB = W // NPHASE    # x-block width (32)
XBP = XB * P        # free els per (arr, phase) block (1024)
GP = 4              # planes per bro/src matmul group (N = GP*128 = 512)


def _compute_sample_coords(mpi_planes, pose_tgt, intrins_src, intrins_tgt):
    """Exact reference math for sample coords, float64. -> ix, iy (P,S,H,W)."""
    Kinv = np.linalg.inv(intrins_tgt.astype(np.float64))
    gx, gy = np.meshgrid(
        np.arange(W, dtype=np.float64), np.arange(H, dtype=np.float64)
    )
    pix = np.stack([gx.ravel(), gy.ravel(), np.ones(H * W)])  # (3, HW)
    cam_dir = Kinv @ pix  # (3, HW)
    ix = np.empty((P, S, H, W))
    iy = np.empty((P, S, H, W))
    for s in range(S):
        K4 = np.zeros((4, 4))
        K4[:3, :3] = intrins_src[s].astype(np.float64)
        K4[3, 3] = 1.0
        proj = K4 @ pose_tgt[s].astype(np.float64)
        for p in range(P):
            cam = np.concatenate(
                [cam_dir * np.float64(mpi_planes[p]), np.ones((1, H * W))], 0
            )
            upc = proj @ cam
            z = upc[2] + 1e-10
            ix[p, s] = (upc[0] / z).reshape(H, W)
            iy[p, s] = (upc[1] / z).reshape(H, W)
    return ix, iy


def _bilinear_matrix(coord_1d, n_in):
    """1D resample matrix M[out, in] with reference tap/validity semantics."""
    n_out = coord_1d.shape[0]
    M = np.zeros((n_out, n_in), np.float64)
    c0 = np.floor(coord_1d)
    w1 = coord_1d - c0
    w0 = 1.0 - w1
    for o in range(n_out):
        i0 = int(c0[o])
        if 0 <= i0 <= n_in - 1:
            M[o, i0] += w0[o]
        if 0 <= i0 + 1 <= n_in - 1:
            M[o, i0 + 1] += w1[o]
    return M


def _reference_numpy(colors, alphas, imgs_src, mpi_planes, pose_tgt,
                     intrins_src, intrins_tgt):
    """Pure-numpy replica of the reference (generic fallback)."""
    Pn, Sn, Hh, Ww = alphas.shape
    ca = 1.0 - alphas
    pm = colors * alphas[..., None]
    overs = np.empty_like(pm)
    over = np.zeros_like(pm[0])
    for d in range(Pn):
        over = over * ca[d][..., None] + pm[d]
        overs[d] = over
    acc = overs[np.maximum(np.arange(Pn) - 2, 0)]
    bro = np.broadcast_to(overs[-1][None], (Pn, Sn, Hh, Ww, 3))
    rc = np.cumprod(ca[::-1], axis=0)[::-1]
    T = np.concatenate([rc[1:], np.ones_like(rc[:1])], axis=0)
    src = np.broadcast_to(imgs_src[None], (Pn, Sn, Hh, Ww, 3))
    stacked = np.concatenate([T[..., None], acc, bro, src], axis=-1)

    ix, iy = _compute_sample_coords(mpi_planes, pose_tgt, intrins_src,
                                    intrins_tgt)
    out = np.empty((Pn, Sn, NCH, Hh, Ww), np.float32)
    for p in range(Pn):
        for s in range(Sn):
            img = stacked[p, s]
            x0 = np.floor(ix[p, s])
            y0 = np.floor(iy[p, s])
            wx1 = ix[p, s] - x0
            wx0 = 1.0 - wx1
            wy1 = iy[p, s] - y0
            wy0 = 1.0 - wy1

            def gather(xx, yy):
                valid = (xx >= 0) & (xx <= Ww - 1) & (yy >= 0) & (yy <= Hh - 1)
                xc = np.clip(xx, 0, Ww - 1).astype(np.int64)
                yc = np.clip(yy, 0, Hh - 1).astype(np.int64)
                return img[yc, xc] * valid[..., None]

            warped = (gather(x0, y0) * (wx0 * wy0)[..., None]
                      + gather(x0 + 1, y0) * (wx1 * wy0)[..., None]
                      + gather(x0, y0 + 1) * (wx0 * wy1)[..., None]
                      + gather(x0 + 1, y0 + 1) * (wx1 * wy1)[..., None])
            out[p, s] = warped.transpose(2, 0, 1).astype(np.float32)
    return out


_CACHED = {}


def _build_bass_program(R):
    """Build the SPMD Bass program (shared by all 8 cores).

    R: y-window height (128 <= R <= 160, multiple of 32); K-chunks
    (128, K1 = R - 128).
    """
    key = ("nc", R)
    if key in _CACHED:
        return _CACHED[key]

    import concourse.bacc as bacc
    import concourse.mybir as mybir
    from concourse import tile

    f32 = mybir.dt.float32
    bf16 = mybir.dt.bfloat16
    MUL = mybir.AluOpType.mult
    ADD = mybir.AluOpType.add

    K1 = R - 128
    NGRP = P // GP
    assert 0 <= K1 <= 32

    nc = bacc.Bacc(
        "TRN2", target_bir_lowering=False, debug=False,
        enable_asserts=False, num_devices=NCORES,
    )

    # ---- DRAM tensors (per-core inputs) ------------------------------
    # compositing data, phase-major; free layout (arr, xb, plane)
    # arrs: 0=ca_over (0 at d=0), 1..3=pm_rgb, 4=ca_T (reversed, 0 at t=0)
    cm_d = nc.dram_tensor("cm", [NPHASE, 128, 5 * XBP], bf16,
                          kind="ExternalInput").ap()
    if K1:
        # ext rows. cols [0:XBP]=ca_over, [XBP:2XBP]=pm_b (partition
        # blocks 0..2 = rgb); cols [2XBP:4XBP]=(ca_T, e_T) at block 0.
        ce_d = nc.dram_tensor("ce", [NPHASE, 128, 4 * XBP], bf16,
                              kind="ExternalInput").ap()
    et_d = nc.dram_tensor("et", [128, XBP], bf16, kind="ExternalInput").ap()
    wy0_d = nc.dram_tensor("wy0", [128, P * 128], bf16,
                           kind="ExternalInput").ap()
    if K1:
        wye_d = nc.dram_tensor("wye", [128, P * 128], bf16,
                               kind="ExternalInput").ap()
        srce_d = nc.dram_tensor("srce", [K1, 3 * W], bf16,
                                kind="ExternalInput").ap()
    srcm_d = nc.dram_tensor("srcm", [128, 3 * W], bf16,
                            kind="ExternalInput").ap()
    # x-warp weights, 8-plane chunks: [ci, xi, (k, dloc, m, xo)]
    wxt_d = nc.dram_tensor("wxt", [4, 128, 2 * 8 * 2 * 128], bf16,
                           kind="ExternalInput").ap()
    # transposed output: [d, xo_in_chunk, (m, ch, yo)]
    out_d = nc.dram_tensor("out", [P, 128, 2 * NCH * 128], bf16,
                           kind="ExternalOutput").ap()

    with tile.TileContext(nc) as tc:
        with (
            tc.tile_pool(name="persist", bufs=1) as persist,
            tc.tile_pool(name="cmp", bufs=2) as cmp_pool,
            tc.tile_pool(name="wx", bufs=2) as wx_pool,
            tc.tile_pool(name="utg", bufs=2) as utg_pool,
            tc.tile_pool(name="utp", bufs=2) as utp_pool,
            tc.tile_pool(name="fsb", bufs=2) as fsb_pool,
            tc.tile_pool(name="ps_g", bufs=2, space="PSUM") as psg,
            tc.tile_pool(name="ps_p", bufs=2, space="PSUM") as psp,
            tc.tile_pool(name="ps_f", bufs=2, space="PSUM") as psf,
        ):
            # ---- persistent tiles -----------------------------------
            # composited channels [row, (ch, x, plane)]; ch 0=T, 1..3=over
            smain = persist.tile([128, 4 * W * P], bf16, tag="smain",
                                 name="smain")
            if K1:
                # ext rows; partition block 32b: b 0..2 = over_{b+1}
                sext = persist.tile([128, W * P], bf16, tag="sext",
                                    name="sext")
                # ext rows, T channel (own tile: matmul operand base
                # partitions are restricted to 0/32/64)
                sext_t = persist.tile([K1, W * P], bf16, tag="sext_t",
                                      name="sext_t")
            et_sb = persist.tile([128, XBP], bf16, tag="et", name="et_sb")
            wy0_sb = persist.tile([128, P * 128], bf16, tag="wy0",
                                  name="wy0_sb")
            if K1:
                wye_sb = persist.tile([128, P * 128], bf16, tag="wye",
                                      name="wye_sb")
                srce_sb = persist.tile([K1, 3 * W], bf16, tag="srce",
                                       name="srce_sb")
            srcm_sb = persist.tile([128, 3 * W], bf16, tag="srcm",
                                   name="srcm_sb")

            nc.sync.dma_start(et_sb[:], et_d[:])
            nc.sync.dma_start(wy0_sb[:], wy0_d[:])
            if K1:
                nc.sync.dma_start(wye_sb[:], wye_d[:])
                nc.sync.dma_start(srce_sb[:], srce_d[:])
            nc.sync.dma_start(srcm_sb[:], srcm_d[:])

            # ---- compositing: DMA + scans per x-phase ----------------
            for f in range(NPHASE):
                cm_t = cmp_pool.tile([128, 5 * XBP], bf16, tag="cm",
                                     name="cm_t")
                nc.sync.dma_start(cm_t[:], cm_d[f])
                if K1:
                    ce_t = cmp_pool.tile([128, 4 * XBP], bf16, tag="ce",
                                         name="ce_t")
                    nc.sync.dma_start(ce_t[:], ce_d[f])

                fo = f * XBP
                # main scans: ch 0..2 on DVE, ch 3 on Pool
                nc.vector.tensor_tensor_scan(
                    smain[:, fo:fo + XBP], cm_t[:, 4 * XBP:5 * XBP],
                    et_sb[:], 0.0, MUL, ADD,
                )
                for c in range(3):
                    eng = nc.vector
                    eng.tensor_tensor_scan(
                        smain[:, (1 + c) * W * P + fo:(1 + c) * W * P + fo
                              + XBP],
                        cm_t[:, 0:XBP], cm_t[:, (1 + c) * XBP:(2 + c) * XBP],
                        0.0, MUL, ADD,
                    )
                if K1:
                    # ext scans (DVE; Pool lacks the scan ISA op)
                    for b in range(3):
                        p0 = 32 * b
                        nc.vector.tensor_tensor_scan(
                            sext[p0:p0 + K1, fo:fo + XBP],
                            ce_t[p0:p0 + K1, 0:XBP],
                            ce_t[p0:p0 + K1, XBP:2 * XBP],
                            0.0, MUL, ADD,
                        )
                    nc.vector.tensor_tensor_scan(
                        sext_t[0:K1, fo:fo + XBP],
                        ce_t[0:K1, 2 * XBP:3 * XBP],
                        ce_t[0:K1, 3 * XBP:4 * XBP],
                        0.0, MUL, ADD,
                    )

            # ---- warps ----------------------------------------------
            # stationary APs: one plane-slot of one channel, stride P
            def s_main_ap(c, q, xc):
                base = c * W * P + xc * 128 * P + q
                return smain[:, base:base + 127 * P + 1:P]

            def s_ext_ap(b, q, xc):
                # b: 0..2 -> over_{b+1} (partition block 32b); 3 -> T
                base = xc * 128 * P + q
                if b == 3:
                    return sext_t[0:K1, base:base + 127 * P + 1:P]
                return sext[32 * b:32 * b + K1, base:base + 127 * P + 1:P]

            def t_slot(d):
                return P - 1 - d

            def acc_slot(d):
                return max(d - 2, 0)

            copy_engs = [nc.scalar, nc.vector]

            def copy_to(eng, dst, src):
                if eng is nc.scalar:
                    eng.copy(dst, src)
                else:
                    eng.tensor_copy(dst, src)

            ncpy = [0]

            def copy_rr(dst, src):
                copy_to(copy_engs[ncpy[0] % 2], dst, src)
                ncpy[0] += 1

            wxt_sb = {}
            for g in range(NGRP):
                ci = g // 2
                if ci not in wxt_sb:
                    t = wx_pool.tile([128, 2 * 8 * 2 * 128], bf16, tag="wx",
                                     name="wx_t")
                    nc.sync.dma_start(t[:], wxt_d[ci])
                    wxt_sb[ci] = t
                dsl = slice(g * GP * 128, (g + 1) * GP * 128)

                # --- bro/src mm1T: 4-plane batched, 6 channels ---
                ut_g = utg_pool.tile([128, GP * 2 * 6 * 128], bf16,
                                     tag="utg", name="ut_g")
                for xc in range(2):
                    for c6 in range(6):
                        if c6 < 3:  # bro: over channel c6+1 at plane 31
                            main_ap = s_main_ap(1 + c6, P - 1, xc)
                            ext_ap = s_ext_ap(c6, P - 1, xc) if K1 else None
                        else:       # src channel c6-3: [row, (c, x)]
                            b0 = (c6 - 3) * W + xc * 128
                            main_ap = srcm_sb[:, b0:b0 + 128]
                            ext_ap = (srce_sb[0:K1, b0:b0 + 128]
                                      if K1 else None)
                        ps_b = psg.tile([128, GP * 128], f32, tag="utg_ps",
                                        name="utg_ps")
                        nc.tensor.matmul(ps_b[:], main_ap, wy0_sb[:, dsl],
                                         start=True, stop=(K1 == 0))
                        if K1:
                            rhs_e = (wye_sb[32 * c6:32 * c6 + K1, dsl]
                                     if c6 < 3 else wye_sb[0:K1, dsl])
                            nc.tensor.matmul(ps_b[:], ext_ap, rhs_e,
                                             start=False, stop=True)
                        # dest [128, (GP, 128)] within [d, k, c6, y] layout
                        dst = ut_g[:].rearrange(
                            "p (d k c y) -> p d k c y", d=GP, k=2, c=6,
                        )[:, :, xc, c6, :]
                        src = ps_b[:].rearrange("p (d y) -> p d y", y=128)
                        copy_rr(dst, src)

                for d in range(g * GP, (g + 1) * GP):
                    # --- T/acc mm1T (per plane) ---
                    ps_p = psp.tile([128, 2 * 4 * 128], f32, tag="utp_ps",
                                    name="utp_ps")
                    rhs_m = wy0_sb[:, d * 128:(d + 1) * 128]
                    rhs_e3 = (wye_sb[0:K1, d * 128:(d + 1) * 128]
                              if K1 else None)
                    for xc in range(2):
                        for ch in range(4):
                            col = xc * 512 + ch * 128
                            if ch == 0:
                                m_ap = s_main_ap(0, t_slot(d), xc)
                                e_ap = (s_ext_ap(3, t_slot(d), xc)
                                        if K1 else None)
                                r_e = rhs_e3
                            else:
                                m_ap = s_main_ap(ch, acc_slot(d), xc)
                                e_ap = (s_ext_ap(ch - 1, acc_slot(d), xc)
                                        if K1 else None)
                                r_e = (wye_sb[32 * (ch - 1):
                                              32 * (ch - 1) + K1,
                                              d * 128:(d + 1) * 128]
                                       if K1 else None)
                            nc.tensor.matmul(
                                ps_p[:, col:col + 128], m_ap, rhs_m,
                                start=True, stop=(K1 == 0),
                            )
                            if K1:
                                nc.tensor.matmul(
                                    ps_p[:, col:col + 128], e_ap, r_e,
                                    start=False, stop=True,
                                )
                    ut_p = utp_pool.tile([128, 2 * 4 * 128], bf16, tag="utp",
                                         name="ut_p")
                    copy_rr(ut_p[:], ps_p[:])
                    utp_v = ut_p[:].rearrange("p (k c y) -> p k c y", k=2,
                                              c=4)
                    utg_v = ut_g[:].rearrange("p (d k c y) -> p d k c y",
                                              d=GP, k=2, c=6)

                    # --- mm2: F^T chunks = wxt^T @ UT ---
                    f_sb = fsb_pool.tile([128, 2 * NCH * 128], bf16,
                                         tag="fsb", name="f_sb")
                    wxv = wxt_sb[ci][:].rearrange(
                        "p (k dl m c) -> p k dl m c", k=2, dl=8, m=2)
                    dloc = d - ci * 8
                    for m in range(2):
                        chunks = [
                            (lambda k: utp_v[:, k, :, :], 512, 0),
                            (lambda k: utg_v[:, d % GP, k, 0:4, :], 512,
                             512),
                            (lambda k: utg_v[:, d % GP, k, 4:6, :], 256,
                             1024),
                        ]
                        for rhs_fn, wdt, fcol in chunks:
                            ps_f = psf.tile([128, 512], f32, tag="f_ps",
                                            name="f_ps")
                            for k in range(2):
                                nc.tensor.matmul(
                                    ps_f[:, 0:wdt],
                                    wxv[:, k, dloc, m, :], rhs_fn(k),
                                    start=(k == 0), stop=(k == 1),
                                )
                            col = m * NCH * 128 + fcol
                            copy_rr(f_sb[:, col:col + wdt], ps_f[:, 0:wdt])
                    # --- out DMA (spread issue across sequencers) ---
                    (nc.sync, nc.scalar)[d % 2].dma_start(
                        out_d[d], f_sb[:])

    nc.compile()
    _CACHED[key] = nc
    return nc


def _host_prepare(colors, alphas, imgs_src, mpi_planes, pose_tgt,
                  intrins_src, intrins_tgt):
    """Build per-core input maps. Returns (in_maps, R, separable)."""
    import ml_dtypes

    bf = ml_dtypes.bfloat16

    ix, iy = _compute_sample_coords(mpi_planes, pose_tgt, intrins_src,
                                    intrins_tgt)
    dev_x = np.abs(ix - ix[:, :, :1, :]).max()
    dev_y = np.abs(iy - iy[:, :, :, :1]).max()
    if dev_x > 1e-3 or dev_y > 1e-3:
        return None, 0, False

    ix1 = ix[:, :, 0, :]  # (P, S, W)
    iy1 = iy[:, :, :, 0]  # (P, S, H)

    My = {}
    Mx = {}
    r_lo = np.full(NCORES, H, np.int64)
    r_hi = np.full(NCORES, -1, np.int64)
    for core in range(NCORES):
        s, h = divmod(core, 2)
        for d in range(P):
            m = _bilinear_matrix(iy1[d, s, h * 128:(h + 1) * 128], H)
            My[(core, d)] = m.astype(np.float32)
            nz = np.nonzero(m.any(axis=0))[0]
            if nz.size:
                r_lo[core] = min(r_lo[core], nz[0])
                r_hi[core] = max(r_hi[core], nz[-1])
    for s in range(S):
        for d in range(P):
            Mx[(s, d)] = _bilinear_matrix(ix1[d, s], W).astype(np.float32)

    width = int((r_hi - r_lo).max()) + 1
    R = max(128, -(-width // 32) * 32)
    if R > 160:
        # ext-row packing supports K1 <= 32; fall back otherwise
        return None, 0, False
    K1 = R - 128

    ez = (np.arange(P) == 0).astype(np.float32)  # inject pattern

    in_maps = []
    for core in range(NCORES):
        s, h = divmod(core, 2)
        r0 = int(min(max(int(r_lo[core]), 0), H - R))

        al = alphas[:, s, r0:r0 + R, :]                      # (P,R,W)
        co = colors[:, s, r0:r0 + R, :, :]                   # (P,R,W,3)
        ca = 1.0 - al
        pm = co * al[..., None]

        ca_ov = np.ascontiguousarray(ca.transpose(1, 2, 0))  # (R,W,P)
        ca_ov[:, :, 0] = 0.0
        pm_t = pm.transpose(1, 2, 3, 0)                      # (R,W,3,P)
        ca_T = np.zeros((R, W, P), np.float32)
        ca_T[:, :, 1:] = ca.transpose(1, 2, 0)[:, :, -1:0:-1]

        def phased(arr128):  # (128, W, P) -> (NPHASE, 128, XBP)
            return np.ascontiguousarray(
                arr128.reshape(128, NPHASE, XB, P).transpose(1, 0, 2, 3)
            ).reshape(NPHASE, 128, XBP)

        cm = np.empty((NPHASE, 128, 5, XBP), np.float32)
        cm[:, :, 0] = phased(ca_ov[:128])
        for c in range(3):
            cm[:, :, 1 + c] = phased(pm_t[:128, :, c])
        cm[:, :, 4] = phased(ca_T[:128])

        imap = {
            "cm": cm.reshape(NPHASE, 128, 5 * XBP).astype(bf),
            "et": np.ascontiguousarray(np.broadcast_to(
                ez[None, None, :], (128, XB, P))).reshape(
                    128, XBP).astype(bf),
            "srcm": np.ascontiguousarray(
                imgs_src[s, r0:r0 + 128].transpose(2, 0, 1)
                .transpose(1, 0, 2)).reshape(128, 3 * W).astype(bf),
        }

        if K1:
            def phased_e(arrk):  # (K1, W, P) -> (NPHASE, K1, XBP)
                return np.ascontiguousarray(
                    arrk.reshape(K1, NPHASE, XB, P).transpose(1, 0, 2, 3)
                ).reshape(NPHASE, K1, XBP)

            ce = np.zeros((NPHASE, 128, 4, XBP), np.float32)
            for b in range(3):
                ce[:, 32 * b:32 * b + K1, 0] = phased_e(ca_ov[128:128 + K1])
                ce[:, 32 * b:32 * b + K1, 1] = phased_e(
                    pm_t[128:128 + K1, :, b])
            ce[:, 0:K1, 2] = phased_e(ca_T[128:128 + K1])
            ce[:, 0:K1, 3] = np.broadcast_to(
                ez[None, None, None, :], (NPHASE, K1, XB, P)).reshape(
                    NPHASE, K1, XBP)
            imap["ce"] = ce.reshape(NPHASE, 128, 4 * XBP).astype(bf)
            imap["srce"] = np.ascontiguousarray(
                imgs_src[s, r0 + 128:r0 + R].transpose(0, 2, 1)
            ).reshape(K1, 3 * W).astype(bf)

        wy0 = np.empty((128, P, 128), np.float32)
        for d in range(P):
            wy0[:, d, :] = My[(core, d)][:, r0:r0 + 128].T
        imap["wy0"] = np.ascontiguousarray(wy0).reshape(
            128, P * 128).astype(bf)
        if K1:
            wye = np.zeros((128, P, 128), np.float32)
            for d in range(P):
                blk = My[(core, d)][:, r0 + 128:r0 + R].T  # (K1,128)
                for a in range(4):
                    wye[32 * a:32 * a + K1, d, :] = blk
            imap["wye"] = np.ascontiguousarray(wye).reshape(
                128, P * 128).astype(bf)

        wxt = np.empty((4, 128, 2, 8, 2, 128), np.float32)
        for d in range(P):
            mx = Mx[(s, d)]  # [xo, xi]
            for k in range(2):
                for m in range(2):
                    wxt[d // 8, :, k, d % 8, m, :] = (
                        mx[m * 128:(m + 1) * 128, k * 128:(k + 1) * 128].T)
        imap["wxt"] = np.ascontiguousarray(wxt).reshape(
            4, 128, 2 * 8 * 2 * 128).astype(bf)

        in_maps.append(imap)
    return in_maps, R, True


def kernel(colors, alphas, imgs_src, mpi_planes, pose_tgt, intrins_src,
           intrins_tgt):
    colors = np.asarray(colors, np.float32)
    alphas = np.asarray(alphas, np.float32)
    imgs_src = np.asarray(imgs_src, np.float32)
    mpi_planes = np.asarray(mpi_planes, np.float32)
    pose_tgt = np.asarray(pose_tgt, np.float32)
    intrins_src = np.asarray(intrins_src, np.float32)
    intrins_tgt = np.asarray(intrins_tgt, np.float32)

    in_maps, R, separable = _host_prepare(
        colors, alphas, imgs_src, mpi_planes, pose_tgt, intrins_src,
        intrins_tgt)
    if not separable:
        return _reference_numpy(colors, alphas, imgs_src, mpi_planes,
                                pose_tgt, intrins_src, intrins_tgt)

    from concourse.bass_utils import run_bass_kernel_spmd

    nc = _build_bass_program(R)
    res = run_bass_kernel_spmd(nc, in_maps, core_ids=list(range(NCORES)))
    _CACHED["last_results"] = res

    out = np.empty((P, S, NCH, H, W), np.float32)
    for core in range(NCORES):
        s, h = divmod(core, 2)
        # raw: [P, xo_in_chunk(128), m(2), ch(10), yo(128)]
        raw = np.asarray(res.results[core]["out"], np.float32).reshape(
            P, 128, 2, NCH, 128)
        # -> [P, ch, yo, m, xo_in_chunk]
        out[:, s, :, h * 128:(h + 1) * 128, :] = (
            raw.transpose(0, 3, 4, 2, 1).reshape(P, NCH, 128, W))
    return out


# revision 20
# speedup vs baseline: 1.1817x; 1.0149x over previous
"""MPI compositing + homography warp kernel for Trainium2 (8 NeuronCores).

For each of P=32 fronto-parallel planes and S=4 source images: composite
per-plane channels (net transmittance T, accumulated-over acc, full-over
bro, source image src -> 10 channels), then bilinear-warp each (plane, src)
channel stack by a plane/source-dependent homography. Output (P, S, 10, H, W).

Structure exploited: the target->source homography here has identity
rotation and shared intrinsics, so sample coordinate ix depends only on x
and iy only on y.  The bilinear gather (zero padding) then factorizes
EXACTLY into two small banded matrices applied left/right:

    warped = My @ S @ Mx^T        per (plane, src, channel)

with per-tap validity folded into the weights (built on the host from the
pose inputs).

Kernel architecture (v3), per core = (source s, row-half h):

1. Compositing as segmented scans.  Channel data lives in SBUF as
   [window-row partition, (ch, x, plane)] with plane minor, so the
   cross-plane recurrences run as `tensor_tensor_scan` along a contiguous
   free dim:
       over:  state = ca_d * state + pm_d      (ca zeroed at d=0 -> reset)
       T:     state = ca'_t * state + e_t      (reversed planes, e=1 at t=0)
   One scan instruction covers 32 planes x a 32-col x-block x 128 rows.

2. y-warp as "mm1T": U^T = (S_slice)^T @ Wy with the composited channel
   image as the matmul *stationary* operand (a stride-32 AP picking one
   plane) and Wy moving.  Produces the transposed intermediate without any
   PE transpose and with one PSUM->SBUF copy.  bro/src channels batch 4
   planes per matmul (shared stationary image, N=512).

3. x-warp: F^T chunks = wxt_block^T @ U^T with wxt stationary and
   channel-batched moving data (N<=512).  Output is written transposed
   ([xo, m, ch, yo]) and untransposed on the host.

Everything on-chip is bf16 except PSUM accumulation (f32); DRAM output is
bf16, upcast on the host.  The y-window (the ~128-160 input rows feeding a
core's 128 output rows) is computed from the actual pose at build time.
"""

import sys

import numpy as np

sys.path.insert(0, "/opt/trn_rl_repo")

P, S, H, W = 32, 4, 256, 256
NCORES = 8
NCH = 10
NPHASE = 8          # x-blocks for compositing DMA/scans
XB = W // NPHASE    # x-block width (32)
XBP = XB * P        # free els per (arr, phase) block (1024)
GP = 4              # planes per bro/src matmul group (N = GP*128 = 512)


def _compute_sample_coords(mpi_planes, pose_tgt, intrins_src, intrins_tgt):
    """Exact reference math for sample coords, float64. -> ix, iy (P,S,H,W)."""
    Kinv = np.linalg.inv(intrins_tgt.astype(np.float64))
    gx, gy = np.meshgrid(
        np.arange(W, dtype=np.float64), np.arange(H, dtype=np.float64)
    )
    pix = np.stack([gx.ravel(), gy.ravel(), np.ones(H * W)])  # (3, HW)
    cam_dir = Kinv @ pix  # (3, HW)
    ix = np.empty((P, S, H, W))
    iy = np.empty((P, S, H, W))
    for s in range(S):
        K4 = np.zeros((4, 4))
        K4[:3, :3] = intrins_src[s].astype(np.float64)
        K4[3, 3] = 1.0
        proj = K4 @ pose_tgt[s].astype(np.float64)
        for p in range(P):
            cam = np.concatenate(
                [cam_dir * np.float64(mpi_planes[p]), np.ones((1, H * W))], 0
            )
            upc = proj @ cam
            z = upc[2] + 1e-10
            ix[p, s] = (upc[0] / z).reshape(H, W)
            iy[p, s] = (upc[1] / z).reshape(H, W)
    return ix, iy


def _bilinear_matrix(coord_1d, n_in):
    """1D resample matrix M[out, in] with reference tap/validity semantics."""
    n_out = coord_1d.shape[0]
    M = np.zeros((n_out, n_in), np.float64)
    c0 = np.floor(coord_1d)
    w1 = coord_1d - c0
    w0 = 1.0 - w1
    for o in range(n_out):
        i0 = int(c0[o])
        if 0 <= i0 <= n_in - 1:
            M[o, i0] += w0[o]
        if 0 <= i0 + 1 <= n_in - 1:
            M[o, i0 + 1] += w1[o]
    return M


def _reference_numpy(colors, alphas, imgs_src, mpi_planes, pose_tgt,
                     intrins_src, intrins_tgt):
    """Pure-numpy replica of the reference (generic fallback)."""
    Pn, Sn, Hh, Ww = alphas.shape
    ca = 1.0 - alphas
    pm = colors * alphas[..., None]
    overs = np.empty_like(pm)
    over = np.zeros_like(pm[0])
    for d in range(Pn):
        over = over * ca[d][..., None] + pm[d]
        overs[d] = over
    acc = overs[np.maximum(np.arange(Pn) - 2, 0)]
    bro = np.broadcast_to(overs[-1][None], (Pn, Sn, Hh, Ww, 3))
    rc = np.cumprod(ca[::-1], axis=0)[::-1]
    T = np.concatenate([rc[1:], np.ones_like(rc[:1])], axis=0)
    src = np.broadcast_to(imgs_src[None], (Pn, Sn, Hh, Ww, 3))
    stacked = np.concatenate([T[..., None], acc, bro, src], axis=-1)

    ix, iy = _compute_sample_coords(mpi_planes, pose_tgt, intrins_src,
                                    intrins_tgt)
    out = np.empty((Pn, Sn, NCH, Hh, Ww), np.float32)
    for p in range(Pn):
        for s in range(Sn):
            img = stacked[p, s]
            x0 = np.floor(ix[p, s])
            y0 = np.floor(iy[p, s])
            wx1 = ix[p, s] - x0
            wx0 = 1.0 - wx1
            wy1 = iy[p, s] - y0
            wy0 = 1.0 - wy1

            def gather(xx, yy):
                valid = (xx >= 0) & (xx <= Ww - 1) & (yy >= 0) & (yy <= Hh - 1)
                xc = np.clip(xx, 0, Ww - 1).astype(np.int64)
                yc = np.clip(yy, 0, Hh - 1).astype(np.int64)
                return img[yc, xc] * valid[..., None]

            warped = (gather(x0, y0) * (wx0 * wy0)[..., None]
                      + gather(x0 + 1, y0) * (wx1 * wy0)[..., None]
                      + gather(x0, y0 + 1) * (wx0 * wy1)[..., None]
                      + gather(x0 + 1, y0 + 1) * (wx1 * wy1)[..., None])
            out[p, s] = warped.transpose(2, 0, 1).astype(np.float32)
    return out


_CACHED = {}


def _build_bass_program(R):
    """Build the SPMD Bass program (shared by all 8 cores).

    R: y-window height (128 <= R <= 160, multiple of 32); K-chunks
    (128, K1 = R - 128).
    """
    key = ("nc", R)
    if key in _CACHED:
        return _CACHED[key]

    import concourse.bacc as bacc
    import concourse.mybir as mybir
    from concourse import tile

    f32 = mybir.dt.float32
    bf16 = mybir.dt.bfloat16
    MUL = mybir.AluOpType.mult
    ADD = mybir.AluOpType.add

    K1 = R - 128
    NGRP = P // GP
    assert 0 <= K1 <= 32

    nc = bacc.Bacc(
        "TRN2", target_bir_lowering=False, debug=False,
        enable_asserts=False, num_devices=NCORES,
    )

    # ---- DRAM tensors (per-core inputs) ------------------------------
    # compositing data, phase-major; free layout (arr, xb, plane)
    # arrs: 0=ca_over (0 at d=0), 1..3=pm_rgb, 4=ca_T (reversed, 0 at t=0)
    cm_d = nc.dram_tensor("cm", [NPHASE, 128, 5 * XBP], bf16,
                          kind="ExternalInput").ap()
    if K1:
        # ext rows. cols [0:XBP]=ca_over, [XBP:2XBP]=pm_b (partition
        # blocks 0..2 = rgb); cols [2XBP:4XBP]=(ca_T, e_T) at block 0.
        ce_d = nc.dram_tensor("ce", [NPHASE, 128, 4 * XBP], bf16,
                              kind="ExternalInput").ap()
    et_d = nc.dram_tensor("et", [128, XBP], bf16, kind="ExternalInput").ap()
    wy0_d = nc.dram_tensor("wy0", [128, P * 128], bf16,
                           kind="ExternalInput").ap()
    if K1:
        wye_d = nc.dram_tensor("wye", [128, P * 128], bf16,
                               kind="ExternalInput").ap()
        srce_d = nc.dram_tensor("srce", [K1, 3 * W], bf16,
                                kind="ExternalInput").ap()
    srcm_d = nc.dram_tensor("srcm", [128, 3 * W], bf16,
                            kind="ExternalInput").ap()
    # x-warp weights, 8-plane chunks: [ci, xi, (k, dloc, m, xo)]
    wxt_d = nc.dram_tensor("wxt", [4, 128, 2 * 8 * 2 * 128], bf16,
                           kind="ExternalInput").ap()
    # transposed output: [d, xo_in_chunk, (m, ch, yo)]
    out_d = nc.dram_tensor("out", [P, 128, 2 * NCH * 128], bf16,
                           kind="ExternalOutput").ap()

    with tile.TileContext(nc) as tc:
        with (
            tc.tile_pool(name="persist", bufs=1) as persist,
            tc.tile_pool(name="cmp", bufs=2) as cmp_pool,
            tc.tile_pool(name="wx", bufs=2) as wx_pool,
            tc.tile_pool(name="utg", bufs=2) as utg_pool,
            tc.tile_pool(name="utp", bufs=2) as utp_pool,
            tc.tile_pool(name="fsb", bufs=2) as fsb_pool,
            tc.tile_pool(name="ps_g", bufs=2, space="PSUM") as psg,
            tc.tile_pool(name="ps_p", bufs=2, space="PSUM") as psp,
            tc.tile_pool(name="ps_f", bufs=2, space="PSUM") as psf,
        ):
            # ---- persistent tiles -----------------------------------
            # composited channels [row, (ch, x, plane)]; ch 0=T, 1..3=over
            smain = persist.tile([128, 4 * W * P], bf16, tag="smain",
                                 name="smain")
            if K1:
                # ext rows; partition block 32b: b 0..2 = over_{b+1}
                sext = persist.tile([128, W * P], bf16, tag="sext",
                                    name="sext")
                # ext rows, T channel (own tile: matmul operand base
                # partitions are restricted to 0/32/64)
                sext_t = persist.tile([K1, W * P], bf16, tag="sext_t",
                                      name="sext_t")
            et_sb = persist.tile([128, XBP], bf16, tag="et", name="et_sb")
            wy0_sb = persist.tile([128, P * 128], bf16, tag="wy0",
                                  name="wy0_sb")
            if K1:
                wye_sb = persist.tile([128, P * 128], bf16, tag="wye",
                                      name="wye_sb")
                srce_sb = persist.tile([K1, 3 * W], bf16, tag="srce",
                                       name="srce_sb")
            srcm_sb = persist.tile([128, 3 * W], bf16, tag="srcm",
                                   name="srcm_sb")

            nc.sync.dma_start(et_sb[:], et_d[:])
            nc.sync.dma_start(wy0_sb[:], wy0_d[:])
            if K1:
                nc.sync.dma_start(wye_sb[:], wye_d[:])
                nc.sync.dma_start(srce_sb[:], srce_d[:])
            nc.sync.dma_start(srcm_sb[:], srcm_d[:])

            # ---- compositing: DMA + scans per x-phase ----------------
            for f in range(NPHASE):
                cm_t = cmp_pool.tile([128, 5 * XBP], bf16, tag="cm",
                                     name="cm_t")
                nc.sync.dma_start(cm_t[:], cm_d[f])
                if K1:
                    ce_t = cmp_pool.tile([128, 4 * XBP], bf16, tag="ce",
                                         name="ce_t")
                    nc.sync.dma_start(ce_t[:], ce_d[f])

                fo = f * XBP
                # main scans: ch 0..2 on DVE, ch 3 on Pool
                nc.vector.tensor_tensor_scan(
                    smain[:, fo:fo + XBP], cm_t[:, 4 * XBP:5 * XBP],
                    et_sb[:], 0.0, MUL, ADD,
                )
                for c in range(3):
                    eng = nc.vector
                    eng.tensor_tensor_scan(
                        smain[:, (1 + c) * W * P + fo:(1 + c) * W * P + fo
                              + XBP],
                        cm_t[:, 0:XBP], cm_t[:, (1 + c) * XBP:(2 + c) * XBP],
                        0.0, MUL, ADD,
                    )
                if K1:
                    # ext scans (DVE; Pool lacks the scan ISA op). The 3
                    # over channels sit at partition blocks 0/32/64 with
                    # identical free ranges -> one 96-partition scan.
                    nc.vector.tensor_tensor_scan(
                        sext[0:96, fo:fo + XBP],
                        ce_t[0:96, 0:XBP],
                        ce_t[0:96, XBP:2 * XBP],
                        0.0, MUL, ADD,
                    )
                    nc.vector.tensor_tensor_scan(
                        sext_t[0:K1, fo:fo + XBP],
                        ce_t[0:K1, 2 * XBP:3 * XBP],
                        ce_t[0:K1, 3 * XBP:4 * XBP],
                        0.0, MUL, ADD,
                    )

            # ---- warps ----------------------------------------------
            # stationary APs: one plane-slot of one channel, stride P
            def s_main_ap(c, q, xc):
                base = c * W * P + xc * 128 * P + q
                return smain[:, base:base + 127 * P + 1:P]

            def s_ext_ap(b, q, xc):
                # b: 0..2 -> over_{b+1} (partition block 32b); 3 -> T
                base = xc * 128 * P + q
                if b == 3:
                    return sext_t[0:K1, base:base + 127 * P + 1:P]
                return sext[32 * b:32 * b + K1, base:base + 127 * P + 1:P]

            def t_slot(d):
                return P - 1 - d

            def acc_slot(d):
                return max(d - 2, 0)

            copy_engs = [nc.scalar, nc.vector]

            def copy_to(eng, dst, src):
                if eng is nc.scalar:
                    eng.copy(dst, src)
                else:
                    eng.tensor_copy(dst, src)

            ncpy = [0]
            act_only = [False]

            def copy_rr(dst, src):
                eng = (nc.scalar if act_only[0]
                       else copy_engs[ncpy[0] % 2])
                copy_to(eng, dst, src)
                ncpy[0] += 1

            def mm1t_group(dsl, specs, uts, n_ch):
                """Batched mm1T for one GP-plane group.

                specs: list of (main_ap, ext_ap, rhs_ext) per (xc, c)."""
                for xc in range(2):
                    for c, (main_ap, ext_ap, rhs_e) in enumerate(
                            specs(xc)):
                        ps_b = psg.tile([128, GP * 128], f32, tag="utg_ps",
                                        name="utg_ps")
                        nc.tensor.matmul(ps_b[:], main_ap, wy0_sb[:, dsl],
                                         start=True, stop=(K1 == 0))
                        if K1:
                            nc.tensor.matmul(ps_b[:], ext_ap, rhs_e,
                                             start=False, stop=True)
                        dst = uts[:].rearrange(
                            "p (d k c y) -> p d k c y", d=GP, k=2, c=n_ch,
                        )[:, :, xc, c, :]
                        src = ps_b[:].rearrange("p (d y) -> p d y", y=128)
                        copy_rr(dst, src)

            def mm2_chunk(wxv, dloc, m, rhs_fn, wdt, f_t, fcol):
                ps_f = psf.tile([128, 512], f32, tag="f_ps", name="f_ps")
                for k in range(2):
                    nc.tensor.matmul(ps_f[:, 0:wdt], wxv[:, k, dloc, m, :],
                                     rhs_fn(k), start=(k == 0),
                                     stop=(k == 1))
                copy_rr(f_t[:, fcol:fcol + wdt], ps_f[:, 0:wdt])

            out_v = out_d.rearrange("d p (m c) -> d p m c", m=2)

            # ===== pass 1: src channels (no scan dependency) =========
            # PE fills with this while the DVE scan phase runs; copies
            # stay off DVE so they don't contend with the scans.
            act_only[0] = True
            wx1 = {}
            for g in range(NGRP):
                ci = g // 2
                if ci not in wx1:
                    t = wx_pool.tile([128, 2 * 8 * 2 * 128], bf16, tag="wx",
                                     name="wx_t")
                    nc.sync.dma_start(t[:], wxt_d[ci])
                    wx1[ci] = t
                dsl = slice(g * GP * 128, (g + 1) * GP * 128)
                ut_s = utg_pool.tile([128, GP * 2 * 3 * 128], bf16,
                                     tag="uts", name="ut_s")

                def src_specs(xc):
                    for c in range(3):
                        b0 = c * W + xc * 128
                        yield (srcm_sb[:, b0:b0 + 128],
                               srce_sb[0:K1, b0:b0 + 128] if K1 else None,
                               wye_sb[0:K1, dsl] if K1 else None)

                mm1t_group(dsl, src_specs, ut_s, 3)
                uts_v = ut_s[:].rearrange("p (d k c y) -> p d k c y",
                                          d=GP, k=2, c=3)
                wxv = wx1[ci][:].rearrange(
                    "p (k dl m c) -> p k dl m c", k=2, dl=8, m=2)
                for d in range(g * GP, (g + 1) * GP):
                    dloc = d - ci * 8
                    f_t = fsb_pool.tile([128, 2 * 3 * 128], bf16,
                                        tag="fsrc", name="f_src")
                    for m in range(2):
                        mm2_chunk(wxv, dloc, m,
                                  lambda k: uts_v[:, d % GP, k, :, :], 384,
                                  f_t, m * 384)
                    nc.scalar.dma_start(
                        out_v[d][:, :, 896:1280],
                        f_t[:].rearrange("p (m c) -> p m c", m=2))

            # ===== pass 2: T/acc/bro channels (after scans) ==========
            act_only[0] = False
            wx2 = {}
            for g in range(NGRP):
                ci = g // 2
                if ci not in wx2:
                    t = wx_pool.tile([128, 2 * 8 * 2 * 128], bf16, tag="wx",
                                     name="wx_t")
                    nc.sync.dma_start(t[:], wxt_d[ci])
                    wx2[ci] = t
                dsl = slice(g * GP * 128, (g + 1) * GP * 128)
                ut_g = utg_pool.tile([128, GP * 2 * 3 * 128], bf16,
                                     tag="utg", name="ut_g")

                def bro_specs(xc):
                    for c in range(3):
                        yield (s_main_ap(1 + c, P - 1, xc),
                               s_ext_ap(c, P - 1, xc) if K1 else None,
                               wye_sb[32 * c:32 * c + K1, dsl]
                               if K1 else None)

                mm1t_group(dsl, bro_specs, ut_g, 3)
                utg_v = ut_g[:].rearrange("p (d k c y) -> p d k c y",
                                          d=GP, k=2, c=3)
                wxv = wx2[ci][:].rearrange(
                    "p (k dl m c) -> p k dl m c", k=2, dl=8, m=2)

                for d in range(g * GP, (g + 1) * GP):
                    # --- T/acc mm1T (per plane) ---
                    ps_p = psp.tile([128, 2 * 4 * 128], f32, tag="utp_ps",
                                    name="utp_ps")
                    rhs_m = wy0_sb[:, d * 128:(d + 1) * 128]
                    for xc in range(2):
                        for ch in range(4):
                            col = xc * 512 + ch * 128
                            if ch == 0:
                                m_ap = s_main_ap(0, t_slot(d), xc)
                                e_ap = (s_ext_ap(3, t_slot(d), xc)
                                        if K1 else None)
                                r_e = (wye_sb[0:K1,
                                              d * 128:(d + 1) * 128]
                                       if K1 else None)
                            else:
                                m_ap = s_main_ap(ch, acc_slot(d), xc)
                                e_ap = (s_ext_ap(ch - 1, acc_slot(d), xc)
                                        if K1 else None)
                                r_e = (wye_sb[32 * (ch - 1):
                                              32 * (ch - 1) + K1,
                                              d * 128:(d + 1) * 128]
                                       if K1 else None)
                            nc.tensor.matmul(
                                ps_p[:, col:col + 128], m_ap, rhs_m,
                                start=True, stop=(K1 == 0),
                            )
                            if K1:
                                nc.tensor.matmul(
                                    ps_p[:, col:col + 128], e_ap, r_e,
                                    start=False, stop=True,
                                )
                    ut_p = utp_pool.tile([128, 2 * 4 * 128], bf16,
                                         tag="utp", name="ut_p")
                    copy_rr(ut_p[:], ps_p[:])
                    utp_v = ut_p[:].rearrange("p (k c y) -> p k c y", k=2,
                                              c=4)

                    # --- mm2 + out ---
                    dloc = d - ci * 8
                    f_t = fsb_pool.tile([128, 2 * 7 * 128], bf16,
                                        tag="fmain", name="f_main")
                    for m in range(2):
                        mm2_chunk(wxv, dloc, m,
                                  lambda k: utp_v[:, k, :, :], 512,
                                  f_t, m * 896)
                        mm2_chunk(wxv, dloc, m,
                                  lambda k: utg_v[:, d % GP, k, :, :], 384,
                                  f_t, m * 896 + 512)
                    (nc.sync, nc.scalar)[d % 2].dma_start(
                        out_v[d][:, :, 0:896],
                        f_t[:].rearrange("p (m c) -> p m c", m=2))

    nc.compile()
    _CACHED[key] = nc
    return nc


def _host_prepare(colors, alphas, imgs_src, mpi_planes, pose_tgt,
                  intrins_src, intrins_tgt):
    """Build per-core input maps. Returns (in_maps, R, separable)."""
    import ml_dtypes

    bf = ml_dtypes.bfloat16

    ix, iy = _compute_sample_coords(mpi_planes, pose_tgt, intrins_src,
                                    intrins_tgt)
    dev_x = np.abs(ix - ix[:, :, :1, :]).max()
    dev_y = np.abs(iy - iy[:, :, :, :1]).max()
    if dev_x > 1e-3 or dev_y > 1e-3:
        return None, 0, False

    ix1 = ix[:, :, 0, :]  # (P, S, W)
    iy1 = iy[:, :, :, 0]  # (P, S, H)

    My = {}
    Mx = {}
    r_lo = np.full(NCORES, H, np.int64)
    r_hi = np.full(NCORES, -1, np.int64)
    for core in range(NCORES):
        s, h = divmod(core, 2)
        for d in range(P):
            m = _bilinear_matrix(iy1[d, s, h * 128:(h + 1) * 128], H)
            My[(core, d)] = m.astype(np.float32)
            nz = np.nonzero(m.any(axis=0))[0]
            if nz.size:
                r_lo[core] = min(r_lo[core], nz[0])
                r_hi[core] = max(r_hi[core], nz[-1])
    for s in range(S):
        for d in range(P):
            Mx[(s, d)] = _bilinear_matrix(ix1[d, s], W).astype(np.float32)

    width = int((r_hi - r_lo).max()) + 1
    R = max(128, -(-width // 32) * 32)
    if R > 160:
        # ext-row packing supports K1 <= 32; fall back otherwise
        return None, 0, False
    K1 = R - 128

    ez = (np.arange(P) == 0).astype(np.float32)  # inject pattern

    in_maps = []
    for core in range(NCORES):
        s, h = divmod(core, 2)
        r0 = int(min(max(int(r_lo[core]), 0), H - R))

        al = alphas[:, s, r0:r0 + R, :]                      # (P,R,W)
        co = colors[:, s, r0:r0 + R, :, :]                   # (P,R,W,3)
        ca = 1.0 - al
        pm = co * al[..., None]

        ca_ov = np.ascontiguousarray(ca.transpose(1, 2, 0))  # (R,W,P)
        ca_ov[:, :, 0] = 0.0
        pm_t = pm.transpose(1, 2, 3, 0)                      # (R,W,3,P)
        ca_T = np.zeros((R, W, P), np.float32)
        ca_T[:, :, 1:] = ca.transpose(1, 2, 0)[:, :, -1:0:-1]

        def phased(arr128):  # (128, W, P) -> (NPHASE, 128, XBP)
            return np.ascontiguousarray(
                arr128.reshape(128, NPHASE, XB, P).transpose(1, 0, 2, 3)
            ).reshape(NPHASE, 128, XBP)

        cm = np.empty((NPHASE, 128, 5, XBP), np.float32)
        cm[:, :, 0] = phased(ca_ov[:128])
        for c in range(3):
            cm[:, :, 1 + c] = phased(pm_t[:128, :, c])
        cm[:, :, 4] = phased(ca_T[:128])

        imap = {
            "cm": cm.reshape(NPHASE, 128, 5 * XBP).astype(bf),
            "et": np.ascontiguousarray(np.broadcast_to(
                ez[None, None, :], (128, XB, P))).reshape(
                    128, XBP).astype(bf),
            "srcm": np.ascontiguousarray(
                imgs_src[s, r0:r0 + 128].transpose(2, 0, 1)
                .transpose(1, 0, 2)).reshape(128, 3 * W).astype(bf),
        }

        if K1:
            def phased_e(arrk):  # (K1, W, P) -> (NPHASE, K1, XBP)
                return np.ascontiguousarray(
                    arrk.reshape(K1, NPHASE, XB, P).transpose(1, 0, 2, 3)
                ).reshape(NPHASE, K1, XBP)

            ce = np.zeros((NPHASE, 128, 4, XBP), np.float32)
            for b in range(3):
                ce[:, 32 * b:32 * b + K1, 0] = phased_e(ca_ov[128:128 + K1])
                ce[:, 32 * b:32 * b + K1, 1] = phased_e(
                    pm_t[128:128 + K1, :, b])
            ce[:, 0:K1, 2] = phased_e(ca_T[128:128 + K1])
            ce[:, 0:K1, 3] = np.broadcast_to(
                ez[None, None, None, :], (NPHASE, K1, XB, P)).reshape(
                    NPHASE, K1, XBP)
            imap["ce"] = ce.reshape(NPHASE, 128, 4 * XBP).astype(bf)
            imap["srce"] = np.ascontiguousarray(
                imgs_src[s, r0 + 128:r0 + R].transpose(0, 2, 1)
            ).reshape(K1, 3 * W).astype(bf)

        wy0 = np.empty((128, P, 128), np.float32)
        for d in range(P):
            wy0[:, d, :] = My[(core, d)][:, r0:r0 + 128].T
        imap["wy0"] = np.ascontiguousarray(wy0).reshape(
            128, P * 128).astype(bf)
        if K1:
            wye = np.zeros((128, P, 128), np.float32)
            for d in range(P):
                blk = My[(core, d)][:, r0 + 128:r0 + R].T  # (K1,128)
                for a in range(4):
                    wye[32 * a:32 * a + K1, d, :] = blk
            imap["wye"] = np.ascontiguousarray(wye).reshape(
                128, P * 128).astype(bf)

        wxt = np.empty((4, 128, 2, 8, 2, 128), np.float32)
        for d in range(P):
            mx = Mx[(s, d)]  # [xo, xi]
            for k in range(2):
                for m in range(2):
                    wxt[d // 8, :, k, d % 8, m, :] = (
                        mx[m * 128:(m + 1) * 128, k * 128:(k + 1) * 128].T)
        imap["wxt"] = np.ascontiguousarray(wxt).reshape(
            4, 128, 2 * 8 * 2 * 128).astype(bf)

        in_maps.append(imap)
    return in_maps, R, True


def kernel(colors, alphas, imgs_src, mpi_planes, pose_tgt, intrins_src,
           intrins_tgt):
    colors = np.asarray(colors, np.float32)
    alphas = np.asarray(alphas, np.float32)
    imgs_src = np.asarray(imgs_src, np.float32)
    mpi_planes = np.asarray(mpi_planes, np.float32)
    pose_tgt = np.asarray(pose_tgt, np.float32)
    intrins_src = np.asarray(intrins_src, np.float32)
    intrins_tgt = np.asarray(intrins_tgt, np.float32)

    in_maps, R, separable = _host_prepare(
        colors, alphas, imgs_src, mpi_planes, pose_tgt, intrins_src,
        intrins_tgt)
    if not separable:
        return _reference_numpy(colors, alphas, imgs_src, mpi_planes,
                                pose_tgt, intrins_src, intrins_tgt)

    from concourse.bass_utils import run_bass_kernel_spmd

    nc = _build_bass_program(R)
    res = run_bass_kernel_spmd(nc, in_maps, core_ids=list(range(NCORES)))
    _CACHED["last_results"] = res

    out = np.empty((P, S, NCH, H, W), np.float32)
    for core in range(NCORES):
        s, h = divmod(core, 2)
        # raw: [P, xo_in_chunk(128), m(2), ch(10), yo(128)]
        raw = np.asarray(res.results[core]["out"], np.float32).reshape(
            P, 128, 2, NCH, 128)
        # -> [P, ch, yo, m, xo_in_chunk]
        out[:, s, :, h * 128:(h + 1) * 128, :] = (
            raw.transpose(0, 3, 4, 2, 1).reshape(P, NCH, 128, W))
    return out


# revision 31
# speedup vs baseline: 1.2839x; 1.0865x over previous
"""MPI compositing + homography warp kernel for Trainium2 (8 NeuronCores).

For each of P=32 fronto-parallel planes and S=4 source images: composite
per-plane channels (net transmittance T, accumulated-over acc, full-over
bro, source image src -> 10 channels), then bilinear-warp each (plane, src)
channel stack by a plane/source-dependent homography. Output (P, S, 10, H, W).

Structure exploited: the target->source homography here has identity
rotation and shared intrinsics, so sample coordinate ix depends only on x
and iy only on y.  The bilinear gather (zero padding) then factorizes
EXACTLY into two small banded matrices applied left/right:

    warped = My @ S @ Mx^T        per (plane, src, channel)

with per-tap validity folded into the weights (built on the host from the
pose inputs).

Kernel architecture (v3), per core = (source s, row-half h):

1. Compositing as segmented scans.  Channel data lives in SBUF as
   [window-row partition, (ch, x, plane)] with plane minor, so the
   cross-plane recurrences run as `tensor_tensor_scan` along a contiguous
   free dim:
       over:  state = ca_d * state + pm_d      (ca zeroed at d=0 -> reset)
       T:     state = ca'_t * state + e_t      (reversed planes, e=1 at t=0)
   One scan instruction covers 32 planes x a 32-col x-block x 128 rows.

2. y-warp as "mm1T": U^T = (S_slice)^T @ Wy with the composited channel
   image as the matmul *stationary* operand (a stride-32 AP picking one
   plane) and Wy moving.  Produces the transposed intermediate without any
   PE transpose and with one PSUM->SBUF copy.  bro/src channels batch 4
   planes per matmul (shared stationary image, N=512).

3. x-warp: F^T chunks = wxt_block^T @ U^T with wxt stationary and
   channel-batched moving data (N<=512).  Output is written transposed
   ([xo, m, ch, yo]) and untransposed on the host.

Everything on-chip is bf16 except PSUM accumulation (f32); DRAM output is
bf16, upcast on the host.  The y-window (the ~128-160 input rows feeding a
core's 128 output rows) is computed from the actual pose at build time.
"""

import sys

import numpy as np

sys.path.insert(0, "/opt/trn_rl_repo")

P, S, H, W = 32, 4, 256, 256
NCORES = 8
NCH = 10
NPHASE = 8          # x-blocks for compositing DMA/scans
XB = W // NPHASE    # x-block width (32)
XBP = XB * P        # free els per (arr, phase) block (1024)
GP = 4              # planes per bro/src matmul group (N = GP*128 = 512)
NGRP = P // GP


def _compute_sample_coords(mpi_planes, pose_tgt, intrins_src, intrins_tgt):
    """Exact reference math for sample coords, float64. -> ix, iy (P,S,H,W)."""
    Kinv = np.linalg.inv(intrins_tgt.astype(np.float64))
    gx, gy = np.meshgrid(
        np.arange(W, dtype=np.float64), np.arange(H, dtype=np.float64)
    )
    pix = np.stack([gx.ravel(), gy.ravel(), np.ones(H * W)])  # (3, HW)
    cam_dir = Kinv @ pix  # (3, HW)
    ix = np.empty((P, S, H, W))
    iy = np.empty((P, S, H, W))
    for s in range(S):
        K4 = np.zeros((4, 4))
        K4[:3, :3] = intrins_src[s].astype(np.float64)
        K4[3, 3] = 1.0
        proj = K4 @ pose_tgt[s].astype(np.float64)
        for p in range(P):
            cam = np.concatenate(
                [cam_dir * np.float64(mpi_planes[p]), np.ones((1, H * W))], 0
            )
            upc = proj @ cam
            z = upc[2] + 1e-10
            ix[p, s] = (upc[0] / z).reshape(H, W)
            iy[p, s] = (upc[1] / z).reshape(H, W)
    return ix, iy


def _bilinear_matrix(coord_1d, n_in):
    """1D resample matrix M[out, in] with reference tap/validity semantics."""
    n_out = coord_1d.shape[0]
    M = np.zeros((n_out, n_in), np.float64)
    c0 = np.floor(coord_1d)
    w1 = coord_1d - c0
    w0 = 1.0 - w1
    for o in range(n_out):
        i0 = int(c0[o])
        if 0 <= i0 <= n_in - 1:
            M[o, i0] += w0[o]
        if 0 <= i0 + 1 <= n_in - 1:
            M[o, i0 + 1] += w1[o]
    return M


def _reference_numpy(colors, alphas, imgs_src, mpi_planes, pose_tgt,
                     intrins_src, intrins_tgt):
    """Pure-numpy replica of the reference (generic fallback)."""
    Pn, Sn, Hh, Ww = alphas.shape
    ca = 1.0 - alphas
    pm = colors * alphas[..., None]
    overs = np.empty_like(pm)
    over = np.zeros_like(pm[0])
    for d in range(Pn):
        over = over * ca[d][..., None] + pm[d]
        overs[d] = over
    acc = overs[np.maximum(np.arange(Pn) - 2, 0)]
    bro = np.broadcast_to(overs[-1][None], (Pn, Sn, Hh, Ww, 3))
    rc = np.cumprod(ca[::-1], axis=0)[::-1]
    T = np.concatenate([rc[1:], np.ones_like(rc[:1])], axis=0)
    src = np.broadcast_to(imgs_src[None], (Pn, Sn, Hh, Ww, 3))
    stacked = np.concatenate([T[..., None], acc, bro, src], axis=-1)

    ix, iy = _compute_sample_coords(mpi_planes, pose_tgt, intrins_src,
                                    intrins_tgt)
    out = np.empty((Pn, Sn, NCH, Hh, Ww), np.float32)
    for p in range(Pn):
        for s in range(Sn):
            img = stacked[p, s]
            x0 = np.floor(ix[p, s])
            y0 = np.floor(iy[p, s])
            wx1 = ix[p, s] - x0
            wx0 = 1.0 - wx1
            wy1 = iy[p, s] - y0
            wy0 = 1.0 - wy1

            def gather(xx, yy):
                valid = (xx >= 0) & (xx <= Ww - 1) & (yy >= 0) & (yy <= Hh - 1)
                xc = np.clip(xx, 0, Ww - 1).astype(np.int64)
                yc = np.clip(yy, 0, Hh - 1).astype(np.int64)
                return img[yc, xc] * valid[..., None]

            warped = (gather(x0, y0) * (wx0 * wy0)[..., None]
                      + gather(x0 + 1, y0) * (wx1 * wy0)[..., None]
                      + gather(x0, y0 + 1) * (wx0 * wy1)[..., None]
                      + gather(x0 + 1, y0 + 1) * (wx1 * wy1)[..., None])
            out[p, s] = warped.transpose(2, 0, 1).astype(np.float32)
    return out


_CACHED = {}


def _build_bass_program(R):
    """Build the SPMD Bass program (shared by all 8 cores).

    R: y-window height (128 <= R <= 160, multiple of 32); K-chunks
    (128, K1 = R - 128).
    """
    key = ("nc", R)
    if key in _CACHED:
        return _CACHED[key]

    import concourse.bacc as bacc
    import concourse.mybir as mybir
    from concourse import tile

    f32 = mybir.dt.float32
    bf16 = mybir.dt.bfloat16
    MUL = mybir.AluOpType.mult
    ADD = mybir.AluOpType.add

    K1 = R - 128
    NGRP = P // GP
    assert 0 <= K1 <= 32

    nc = bacc.Bacc(
        "TRN2", target_bir_lowering=False, debug=False,
        enable_asserts=False, num_devices=NCORES,
    )

    # ---- DRAM tensors (per-core inputs) ------------------------------
    # compositing data, phase-major; free layout (arr, xb, plane)
    # arrs: 0=ca_over (0 at d=0), 1..3=pm_rgb, 4=ca_T (reversed, 0 at t=0)
    cm_d = nc.dram_tensor("cm", [NPHASE, 128, 5 * XBP], bf16,
                          kind="ExternalInput").ap()
    if K1:
        # ext rows. cols [0:XBP]=ca_over, [XBP:2XBP]=pm_b (partition
        # blocks 0..2 = rgb); cols [2XBP:4XBP]=(ca_T, e_T) at block 0.
        ce_d = nc.dram_tensor("ce", [NPHASE, 128, 4 * XBP], bf16,
                              kind="ExternalInput").ap()
    et_d = nc.dram_tensor("et", [128, XBP], bf16, kind="ExternalInput").ap()
    wy0_d = nc.dram_tensor("wy0", [128, P * 128], bf16,
                           kind="ExternalInput").ap()
    if K1:
        wye_d = nc.dram_tensor("wye", [128, P * 128], bf16,
                               kind="ExternalInput").ap()
    # host-precomputed src y-warp U^T, per group: [g, xi, (dloc, k, c, yo)]
    uts_d = nc.dram_tensor("uts", [NGRP, 128, GP * 2 * 3 * 128], bf16,
                           kind="ExternalInput").ap()
    # x-warp weights, 8-plane chunks: [ci, xi, (k, dloc, m, xo)]
    wxt_d = nc.dram_tensor("wxt", [4, 128, 2 * 8 * 2 * 128], bf16,
                           kind="ExternalInput").ap()
    # transposed output: [d, xo_in_chunk, (m, ch, yo)]
    out_d = nc.dram_tensor("out", [P, 128, 2 * NCH * 128], bf16,
                           kind="ExternalOutput").ap()

    with tile.TileContext(nc) as tc:
        with (
            tc.tile_pool(name="persist", bufs=1) as persist,
            tc.tile_pool(name="cmp", bufs=2) as cmp_pool,
            tc.tile_pool(name="wx", bufs=2) as wx_pool,
            tc.tile_pool(name="utg", bufs=2) as utg_pool,
            tc.tile_pool(name="utp", bufs=2) as utp_pool,
            tc.tile_pool(name="fsb", bufs=2) as fsb_pool,
            tc.tile_pool(name="ps_g", bufs=2, space="PSUM") as psg,
            tc.tile_pool(name="ps_p", bufs=2, space="PSUM") as psp,
            tc.tile_pool(name="ps_f", bufs=2, space="PSUM") as psf,
        ):
            # ---- persistent tiles -----------------------------------
            # composited channels [row, (ch, x, plane)]; ch 0=T, 1..3=over
            smain = persist.tile([128, 4 * W * P], bf16, tag="smain",
                                 name="smain")
            if K1:
                # ext rows; partition block 32b: b 0..2 = over_{b+1}
                sext = persist.tile([128, W * P], bf16, tag="sext",
                                    name="sext")
                # ext rows, T channel (own tile: matmul operand base
                # partitions are restricted to 0/32/64)
                sext_t = persist.tile([K1, W * P], bf16, tag="sext_t",
                                      name="sext_t")
            et_sb = persist.tile([128, XBP], bf16, tag="et", name="et_sb")
            wy0_sb = persist.tile([128, P * 128], bf16, tag="wy0",
                                  name="wy0_sb")
            if K1:
                wye_sb = persist.tile([128, P * 128], bf16, tag="wye",
                                      name="wye_sb")

            nc.sync.dma_start(et_sb[:], et_d[:])
            nc.sync.dma_start(wy0_sb[:], wy0_d[:])
            if K1:
                nc.sync.dma_start(wye_sb[:], wye_d[:])

            # ---- warps ----------------------------------------------
            # stationary APs: one plane-slot of one channel, stride P
            def s_main_ap(c, q, xc):
                base = c * W * P + xc * 128 * P + q
                return smain[:, base:base + 127 * P + 1:P]

            def s_ext_ap(b, q, xc):
                # b: 0..2 -> over_{b+1} (partition block 32b); 3 -> T
                base = xc * 128 * P + q
                if b == 3:
                    return sext_t[0:K1, base:base + 127 * P + 1:P]
                return sext[32 * b:32 * b + K1, base:base + 127 * P + 1:P]

            def t_slot(d):
                return P - 1 - d

            def acc_slot(d):
                return max(d - 2, 0)

            copy_engs = [nc.scalar, nc.vector]

            def copy_to(eng, dst, src):
                if eng is nc.scalar:
                    eng.copy(dst, src)
                else:
                    eng.tensor_copy(dst, src)

            ncpy = [0]
            act_only = [False]

            def copy_rr(dst, src):
                eng = (nc.scalar if act_only[0]
                       else copy_engs[ncpy[0] % 2])
                copy_to(eng, dst, src)
                ncpy[0] += 1

            def mm1t_group(dsl, specs, uts, n_ch):
                """Batched mm1T for one GP-plane group.

                specs: list of (main_ap, ext_ap, rhs_ext) per (xc, c)."""
                for xc in range(2):
                    for c, (main_ap, ext_ap, rhs_e) in enumerate(
                            specs(xc)):
                        ps_b = psg.tile([128, GP * 128], f32, tag="utg_ps",
                                        name="utg_ps")
                        nc.tensor.matmul(ps_b[:], main_ap, wy0_sb[:, dsl],
                                         start=True, stop=(K1 == 0))
                        if K1:
                            nc.tensor.matmul(ps_b[:], ext_ap, rhs_e,
                                             start=False, stop=True)
                        dst = uts[:].rearrange(
                            "p (d k c y) -> p d k c y", d=GP, k=2, c=n_ch,
                        )[:, :, xc, c, :]
                        src = ps_b[:].rearrange("p (d y) -> p d y", y=128)
                        copy_rr(dst, src)

            def mm2_chunk(wxv, dloc, m, rhs_fn, wdt, f_t, fcol):
                ps_f = psf.tile([128, 512], f32, tag="f_ps", name="f_ps")
                for k in range(2):
                    nc.tensor.matmul(ps_f[:, 0:wdt], wxv[:, k, dloc, m, :],
                                     rhs_fn(k), start=(k == 0),
                                     stop=(k == 1))
                copy_rr(f_t[:, fcol:fcol + wdt], ps_f[:, 0:wdt])

            out_v = out_d.rearrange("d p (m c) -> d p m c", m=2)

            # ===== pass 1: src channels (no scan dependency) =========
            # The src y-warp U^T is host-precomputed and DMA'd in; PE
            # fills with the src x-warp while the DVE scan phase runs.
            # Copies stay off DVE so they don't contend with the scans.
            act_only[0] = True
            wx1 = {}
            for g in range(NGRP):
                ci = g // 2
                if ci not in wx1:
                    t = wx_pool.tile([128, 2 * 8 * 2 * 128], bf16, tag="wx",
                                     name="wx_t")
                    nc.sync.dma_start(t[:], wxt_d[ci])
                    wx1[ci] = t
                ut_s = utg_pool.tile([128, GP * 2 * 3 * 128], bf16,
                                     tag="uts", name="ut_s")
                nc.sync.dma_start(ut_s[:], uts_d[g])
                uts_v = ut_s[:].rearrange("p (d k c y) -> p d k c y",
                                          d=GP, k=2, c=3)
                wxv = wx1[ci][:].rearrange(
                    "p (k dl m c) -> p k dl m c", k=2, dl=8, m=2)
                for d in range(g * GP, (g + 1) * GP):
                    dloc = d - ci * 8
                    f_t = fsb_pool.tile([128, 2 * 7 * 128], bf16,
                                        tag="fmain", name="f_src")
                    for m in range(2):
                        mm2_chunk(wxv, dloc, m,
                                  lambda k: uts_v[:, d % GP, k, :, :], 384,
                                  f_t, m * 384)
                    nc.scalar.dma_start(
                        out_v[d][:, :, 896:1280],
                        f_t[:, 0:768].rearrange("p (m c) -> p m c", m=2))

            # ===== compositing: DMA (Pool sequencer) + scans (DVE) ====
            for f in range(NPHASE):
                cm_t = cmp_pool.tile([128, 5 * XBP], bf16, tag="cm",
                                     name="cm_t")
                nc.gpsimd.dma_start(cm_t[:], cm_d[f])
                if K1:
                    ce_t = cmp_pool.tile([128, 4 * XBP], bf16, tag="ce",
                                         name="ce_t")
                    nc.gpsimd.dma_start(ce_t[:], ce_d[f])

                fo = f * XBP
                nc.vector.tensor_tensor_scan(
                    smain[:, fo:fo + XBP], cm_t[:, 4 * XBP:5 * XBP],
                    et_sb[:], 0.0, MUL, ADD,
                )
                for c in range(3):
                    nc.vector.tensor_tensor_scan(
                        smain[:, (1 + c) * W * P + fo:(1 + c) * W * P + fo
                              + XBP],
                        cm_t[:, 0:XBP], cm_t[:, (1 + c) * XBP:(2 + c) * XBP],
                        0.0, MUL, ADD,
                    )
                if K1:
                    # ext scans: the 3 over channels sit at partition
                    # blocks 0/32/64 with identical free ranges -> one
                    # 96-partition scan; T ext separate.
                    nc.vector.tensor_tensor_scan(
                        sext[0:96, fo:fo + XBP],
                        ce_t[0:96, 0:XBP],
                        ce_t[0:96, XBP:2 * XBP],
                        0.0, MUL, ADD,
                    )
                    nc.vector.tensor_tensor_scan(
                        sext_t[0:K1, fo:fo + XBP],
                        ce_t[0:K1, 2 * XBP:3 * XBP],
                        ce_t[0:K1, 3 * XBP:4 * XBP],
                        0.0, MUL, ADD,
                    )

            # ===== pass 2: T/acc/bro channels (after scans) ==========
            act_only[0] = False
            wx2 = {}
            for g in range(NGRP):
                ci = g // 2
                if ci not in wx2:
                    t = wx_pool.tile([128, 2 * 8 * 2 * 128], bf16, tag="wx",
                                     name="wx_t")
                    nc.sync.dma_start(t[:], wxt_d[ci])
                    wx2[ci] = t
                dsl = slice(g * GP * 128, (g + 1) * GP * 128)
                ut_g = utg_pool.tile([128, GP * 2 * 3 * 128], bf16,
                                     tag="utg", name="ut_g")

                def bro_specs(xc):
                    for c in range(3):
                        yield (s_main_ap(1 + c, P - 1, xc),
                               s_ext_ap(c, P - 1, xc) if K1 else None,
                               wye_sb[32 * c:32 * c + K1, dsl]
                               if K1 else None)

                mm1t_group(dsl, bro_specs, ut_g, 3)
                utg_v = ut_g[:].rearrange("p (d k c y) -> p d k c y",
                                          d=GP, k=2, c=3)
                wxv = wx2[ci][:].rearrange(
                    "p (k dl m c) -> p k dl m c", k=2, dl=8, m=2)

                sm_v = smain[:].rearrange("r (c x q) -> r c x q", c=4, x=W)
                se_v = (sext[:].rearrange("r (x q) -> r x q", x=W)
                        if K1 else None)
                st_v = (sext_t[:].rearrange("r (x q) -> r x q", x=W)
                        if K1 else None)

                for d in range(g * GP, (g + 1) * GP):
                    # --- T/acc y-warp: chain mm1 (wy stationary, channel
                    # batched) -> U [yo, (a1 a2 T a3) x 256x] in PSUM ---
                    ps_p = psp.tile([128, 2 * 4 * 128], f32, tag="utp_ps",
                                    name="utp_ps")
                    lhs_m = wy0_sb[:, d * 128:(d + 1) * 128]
                    qa, qt = acc_slot(d), t_slot(d)
                    # U column order: (a1, a2, T, a3) x 256 each.
                    # (rhs_main, rhs_ext, wye base, col)
                    parts = [
                        (sm_v[:, 1, :, qa], se_v[0:K1, :, qa]
                         if K1 else None, 0, 0),
                        (sm_v[:, 2, :, qa], se_v[32:32 + K1, :, qa]
                         if K1 else None, 32, 256),
                        (sm_v[:, 0, :, qt], st_v[0:K1, :, qt]
                         if K1 else None, 0, 512),
                        (sm_v[:, 3, :, qa], se_v[64:64 + K1, :, qa]
                         if K1 else None, 64, 768),
                    ]
                    for rhs_m_ap, rhs_e_ap, eb, col in parts:
                        nc.tensor.matmul(
                            ps_p[:, col:col + 256], lhs_m, rhs_m_ap,
                            start=True, stop=(K1 == 0),
                        )
                        if K1:
                            lhs_e = wye_sb[eb:eb + K1,
                                           d * 128:(d + 1) * 128]
                            nc.tensor.matmul(
                                ps_p[:, col:col + 256], lhs_e, rhs_e_ap,
                                start=False, stop=True,
                            )
                    u_sb = utp_pool.tile([128, 1024], bf16, tag="usb",
                                         name="u_sb")
                    nc.scalar.copy(u_sb[:], ps_p[:])
                    ut_p = utp_pool.tile([128, 1024], bf16, tag="utp",
                                         name="ut_p")
                    # XBAR transpose: ut_p[p, j, r] = u_sb[r, 128j + p]
                    (nc.sync, nc.scalar)[d % 2].dma_start(
                        ut_p[:].rearrange("p (j y) -> p j y", j=8),
                        u_sb[:], transpose=True)
                    utp_v = ut_p[:].rearrange("p (j y) -> p j y", j=8)

                    # --- mm2 + out ---
                    dloc = d - ci * 8
                    f_t = fsb_pool.tile([128, 2 * 7 * 128], bf16,
                                        tag="fmain", name="f_main")
                    for m in range(2):
                        mm2_chunk(wxv, dloc, m,
                                  lambda k: utp_v[:, k:8:2, :], 512,
                                  f_t, m * 896)
                        mm2_chunk(wxv, dloc, m,
                                  lambda k: utg_v[:, d % GP, k, :, :], 384,
                                  f_t, m * 896 + 512)
                    (nc.sync, nc.scalar)[d % 2].dma_start(
                        out_v[d][:, :, 0:896],
                        f_t[:].rearrange("p (m c) -> p m c", m=2))

    nc.compile()
    _CACHED[key] = nc
    return nc


def _host_prepare(colors, alphas, imgs_src, mpi_planes, pose_tgt,
                  intrins_src, intrins_tgt):
    """Build per-core input maps. Returns (in_maps, R, separable)."""
    import ml_dtypes

    bf = ml_dtypes.bfloat16

    ix, iy = _compute_sample_coords(mpi_planes, pose_tgt, intrins_src,
                                    intrins_tgt)
    dev_x = np.abs(ix - ix[:, :, :1, :]).max()
    dev_y = np.abs(iy - iy[:, :, :, :1]).max()
    if dev_x > 1e-3 or dev_y > 1e-3:
        return None, 0, False

    ix1 = ix[:, :, 0, :]  # (P, S, W)
    iy1 = iy[:, :, :, 0]  # (P, S, H)

    My = {}
    Mx = {}
    r_lo = np.full(NCORES, H, np.int64)
    r_hi = np.full(NCORES, -1, np.int64)
    for core in range(NCORES):
        s, h = divmod(core, 2)
        for d in range(P):
            m = _bilinear_matrix(iy1[d, s, h * 128:(h + 1) * 128], H)
            My[(core, d)] = m.astype(np.float32)
            nz = np.nonzero(m.any(axis=0))[0]
            if nz.size:
                r_lo[core] = min(r_lo[core], nz[0])
                r_hi[core] = max(r_hi[core], nz[-1])
    for s in range(S):
        for d in range(P):
            Mx[(s, d)] = _bilinear_matrix(ix1[d, s], W).astype(np.float32)

    width = int((r_hi - r_lo).max()) + 1
    R = max(128, -(-width // 32) * 32)
    if R > 160:
        # ext-row packing supports K1 <= 32; fall back otherwise
        return None, 0, False
    K1 = R - 128

    ez = (np.arange(P) == 0).astype(np.float32)  # inject pattern

    in_maps = []
    for core in range(NCORES):
        s, h = divmod(core, 2)
        r0 = int(min(max(int(r_lo[core]), 0), H - R))

        al = alphas[:, s, r0:r0 + R, :]                      # (P,R,W)
        co = colors[:, s, r0:r0 + R, :, :]                   # (P,R,W,3)
        ca = 1.0 - al
        pm = co * al[..., None]

        ca_ov = np.ascontiguousarray(ca.transpose(1, 2, 0))  # (R,W,P)
        ca_ov[:, :, 0] = 0.0
        pm_t = pm.transpose(1, 2, 3, 0)                      # (R,W,3,P)
        ca_T = np.zeros((R, W, P), np.float32)
        ca_T[:, :, 1:] = ca.transpose(1, 2, 0)[:, :, -1:0:-1]

        def phased(arr128):  # (128, W, P) -> (NPHASE, 128, XBP)
            return np.ascontiguousarray(
                arr128.reshape(128, NPHASE, XB, P).transpose(1, 0, 2, 3)
            ).reshape(NPHASE, 128, XBP)

        cm = np.empty((NPHASE, 128, 5, XBP), np.float32)
        cm[:, :, 0] = phased(ca_ov[:128])
        for c in range(3):
            cm[:, :, 1 + c] = phased(pm_t[:128, :, c])
        cm[:, :, 4] = phased(ca_T[:128])

        # host-precomputed src y-warp: U_src[d,c] = My_d @ imgs_src[s,..,c]
        my_all = np.stack([My[(core, d)] for d in range(P)])  # (P,128,H)
        u_src = np.einsum('dor,rxc->doxc', my_all,
                          imgs_src[s].astype(np.float32))  # (P,128yo,W,3)
        # -> uts[g, p, dloc, k, c, yo] = u_src[d, yo, k*128+p, c]
        uts = np.ascontiguousarray(
            u_src.reshape(NGRP, GP, 128, 2, 128, 3)
            .transpose(0, 4, 1, 3, 5, 2)).reshape(
                NGRP, 128, GP * 2 * 3 * 128)

        imap = {
            "cm": cm.reshape(NPHASE, 128, 5 * XBP).astype(bf),
            "et": np.ascontiguousarray(np.broadcast_to(
                ez[None, None, :], (128, XB, P))).reshape(
                    128, XBP).astype(bf),
            "uts": uts.astype(bf),
        }

        if K1:
            def phased_e(arrk):  # (K1, W, P) -> (NPHASE, K1, XBP)
                return np.ascontiguousarray(
                    arrk.reshape(K1, NPHASE, XB, P).transpose(1, 0, 2, 3)
                ).reshape(NPHASE, K1, XBP)

            ce = np.zeros((NPHASE, 128, 4, XBP), np.float32)
            for b in range(3):
                ce[:, 32 * b:32 * b + K1, 0] = phased_e(ca_ov[128:128 + K1])
                ce[:, 32 * b:32 * b + K1, 1] = phased_e(
                    pm_t[128:128 + K1, :, b])
            ce[:, 0:K1, 2] = phased_e(ca_T[128:128 + K1])
            ce[:, 0:K1, 3] = np.broadcast_to(
                ez[None, None, None, :], (NPHASE, K1, XB, P)).reshape(
                    NPHASE, K1, XBP)
            imap["ce"] = ce.reshape(NPHASE, 128, 4 * XBP).astype(bf)

        wy0 = np.empty((128, P, 128), np.float32)
        for d in range(P):
            wy0[:, d, :] = My[(core, d)][:, r0:r0 + 128].T
        imap["wy0"] = np.ascontiguousarray(wy0).reshape(
            128, P * 128).astype(bf)
        if K1:
            wye = np.zeros((128, P, 128), np.float32)
            for d in range(P):
                blk = My[(core, d)][:, r0 + 128:r0 + R].T  # (K1,128)
                for a in range(4):
                    wye[32 * a:32 * a + K1, d, :] = blk
            imap["wye"] = np.ascontiguousarray(wye).reshape(
                128, P * 128).astype(bf)

        wxt = np.empty((4, 128, 2, 8, 2, 128), np.float32)
        for d in range(P):
            mx = Mx[(s, d)]  # [xo, xi]
            for k in range(2):
                for m in range(2):
                    wxt[d // 8, :, k, d % 8, m, :] = (
                        mx[m * 128:(m + 1) * 128, k * 128:(k + 1) * 128].T)
        imap["wxt"] = np.ascontiguousarray(wxt).reshape(
            4, 128, 2 * 8 * 2 * 128).astype(bf)

        in_maps.append(imap)
    return in_maps, R, True


def kernel(colors, alphas, imgs_src, mpi_planes, pose_tgt, intrins_src,
           intrins_tgt):
    colors = np.asarray(colors, np.float32)
    alphas = np.asarray(alphas, np.float32)
    imgs_src = np.asarray(imgs_src, np.float32)
    mpi_planes = np.asarray(mpi_planes, np.float32)
    pose_tgt = np.asarray(pose_tgt, np.float32)
    intrins_src = np.asarray(intrins_src, np.float32)
    intrins_tgt = np.asarray(intrins_tgt, np.float32)

    in_maps, R, separable = _host_prepare(
        colors, alphas, imgs_src, mpi_planes, pose_tgt, intrins_src,
        intrins_tgt)
    if not separable:
        return _reference_numpy(colors, alphas, imgs_src, mpi_planes,
                                pose_tgt, intrins_src, intrins_tgt)

    from concourse.bass_utils import run_bass_kernel_spmd

    nc = _build_bass_program(R)
    res = run_bass_kernel_spmd(nc, in_maps, core_ids=list(range(NCORES)))
    _CACHED["last_results"] = res

    # device channel order (a1, a2, T, a3, b*, s*) -> reference order
    jorder = [2, 0, 1, 3, 4, 5, 6, 7, 8, 9]
    out = np.empty((P, S, NCH, H, W), np.float32)
    for core in range(NCORES):
        s, h = divmod(core, 2)
        # raw: [P, xo_in_chunk(128), m(2), ch(10), yo(128)]
        raw = np.asarray(res.results[core]["out"], np.float32).reshape(
            P, 128, 2, NCH, 128)[:, :, :, jorder, :]
        # -> [P, ch, yo, m, xo_in_chunk]
        out[:, s, :, h * 128:(h + 1) * 128, :] = (
            raw.transpose(0, 3, 4, 2, 1).reshape(P, NCH, 128, W))
    return out
